# revision 1
# baseline (speedup 1.0000x reference)
"""Trainium2 Bass kernel for the GIN message-passing model (8 NeuronCores).

Sharding: graph partitioning.  Core c owns graphs [c*G/8, (c+1)*G/8) and the
contiguous node range of those graphs (batch is sorted), plus every edge whose
dst lands there (+ synthetic self-edges folding the GIN "+h" term into the
aggregation).  dst nodes get compact slot ranks.

Aggregation: edges are gathered with `dma_gather` (int16 indices, so the
source row space is split into 4 ranges => 4 passes).  Within a pass edges
are dst-sorted and packed into 128-position chunks aligned to 128-slot
"subbins"; a chunk's segment-sum is one matmul (gathered rows as stationary,
an on-chip-generated one-hot as moving operand) into the subbin's slice of a
512-slot "bin" PSUM bank.  Each (bin, pass) accumulates in PSUM, then one DVE
add folds it into the z accumulator in SBUF.  BatchNorm stats AllReduce; h1
is stored node-major (via PE transposes) and AllGathered for conv2's gather;
pooling is windowed one-hot matmuls; the MLP head runs feature-major.
"""

import sys

for _p in ("/opt/trn_rl_repo",):
    if _p not in sys.path:
        sys.path.insert(0, _p)

import numpy as np
from contextlib import ExitStack

import concourse.bass as bass
import concourse.bacc as bacc
import concourse.mybir as mybir
import concourse.tile as tile
from concourse.bass_utils import run_bass_kernel_spmd
from concourse.tile_rust import add_dep_helper

F32 = mybir.dt.float32
BF16 = mybir.dt.bfloat16
I32 = mybir.dt.int32
I16 = mybir.dt.int16
AF = mybir.ActivationFunctionType
ALU = mybir.AluOpType

BN_EPS = 1e-5
PADCOL = 200.0          # colidx value for pad positions (never matches 0..127)


class Cfg:
    def __init__(self, N=100000, E=500000, G=2048, D=128, OUT=64, FIN=2, W=8,
                 NR=4, NIMAX=4096, GW=32, GDT=BF16, DBG=99):
        self.N, self.E, self.G, self.D, self.OUT, self.FIN, self.W = N, E, G, D, OUT, FIN, W
        self.NR = NR        # source ranges (int16 index limit)
        self.NIMAX = NIMAX  # max positions per dma_gather
        self.GW = GW        # pooling window width (graphs)
        self.GDT = GDT      # gather dtype (bf16 or f32)
        self.DBG = DBG      # debug cut level (99 = full program)
        self.GPC = G // W   # graphs per core


DEFAULT_CFG = Cfg()


def _wrap_idx(lst):
    """dma_gather index layout: position j is read from row j%16, col j//16."""
    assert len(lst) % 16 == 0
    return np.tile(np.asarray(lst, np.int16).reshape(-1, 16).T, (8, 1))


# ---------------------------------------------------------------- host plan

def _plan(edge_index, batch, cfg):
    c = cfg
    batch = np.asarray(batch).astype(np.int64)
    ei = np.asarray(edge_index).astype(np.int64)
    owner = (batch // c.GPC).astype(np.int64)

    # self-edges appended
    src2 = np.concatenate([ei[0], np.arange(c.N, dtype=np.int64)])
    dst2 = np.concatenate([ei[1], np.arange(c.N, dtype=np.int64)])
    eowner = owner[dst2]

    # compact slot ranks per core
    n_real = np.zeros(c.W, np.int64)
    slot_of = np.full(c.N, -1, np.int64)
    node_lo = np.zeros(c.W + 1, np.int64)
    for ci in range(c.W):
        node_lo[ci] = np.searchsorted(batch, ci * c.GPC)
    node_lo[c.W] = c.N
    for ci in range(c.W):
        lo, hi = node_lo[ci], node_lo[ci + 1]
        n_real[ci] = hi - lo
        slot_of[lo:hi] = np.arange(hi - lo)
    S = int(((n_real.max() + 511) // 512) * 512)
    assert 2 * S <= 32767, f"S={S} too large for int16 conv2 ranges"
    nbin = S // 512
    nsub = S // 128
    nSC = S // 128
    gslot = owner * S + slot_of

    def build_conv(src_row, R):
        """src_row: per-edge source row id in the gather table (size R).
        Ranges are interleaved (pass = row % NR) so per-core locality in the
        source space cannot overload one pass.  The device view is
        table.rearrange("(q four) f -> four q f")[r] with elem_step.
        Returns common chunk structure + per-core idx/colidx arrays."""
        RSZ = -(-max(R, 1) // c.NR)
        RSZ = ((RSZ + 127) // 128) * 128
        assert RSZ <= 32767
        epass = src_row % c.NR
        # per (core, pass, subbin) edge lists
        counts = np.zeros((c.W, c.NR, nsub), np.int64)
        percore_edges = []
        for ci in range(c.W):
            m = eowner == ci
            sl = slot_of[dst2[m]]
            pr = epass[m]
            rows = src_row[m]
            sub = sl // 128
            order = np.lexsort((sl, sub, pr))
            sl, pr, rows, sub = sl[order], pr[order], rows[order], sub[order]
            np.add.at(counts[ci], (pr, sub), 1)
            percore_edges.append((sl, pr, rows, sub))
        # common chunk structure
        nch = np.maximum(1, -(-counts.max(axis=0) // 128))   # [NR, nsub]
        chunks = []     # (pass, subbin)
        seg_of = {}
        for r in range(c.NR):
            for sb in range(nsub):
                seg_of[(r, sb)] = (len(chunks), int(nch[r, sb]))
                for k in range(int(nch[r, sb])):
                    chunks.append((r, sb))
        C = len(chunks)
        POS = C * 128
        pass_pos_lo = np.zeros(c.NR + 1, np.int64)
        for r in range(c.NR):
            pass_pos_lo[r + 1] = pass_pos_lo[r] + 128 * int(nch[r].sum())
        # per-core arrays
        cores = []
        for ci in range(c.W):
            sl, pr, rows, sub = percore_edges[ci]
            idx_local = np.zeros(POS, np.int64)          # pad -> row 0 of range
            colv = np.full((128, C), PADCOL, np.float64)
            # compute position of each edge: within its (pass, subbin) segment
            seg_base = {}
            cursor = {}
            pos = 0
            for r in range(c.NR):
                for sb in range(nsub):
                    seg_base[(r, sb)] = pos
                    cursor[(r, sb)] = 0
                    pos += 128 * int(nch[r, sb])
            # vectorized-ish placement
            key = pr * nsub + sub
            # edges are sorted by (pr, sub, sl); within segment consecutive
            uniq, start_idx = np.unique(key, return_index=True)
            end_idx = np.append(start_idx[1:], len(key))
            for u, s0, s1 in zip(uniq, start_idx, end_idx):
                r, sb = int(u) // nsub, int(u) % nsub
                base = seg_base[(r, sb)]
                n = s1 - s0
                p = base + np.arange(n)
                idx_local[p] = rows[s0:s1] // c.NR
                colv[p % 128, p // 128] = sl[s0:s1] - sb * 128
            # per-pass wrapped idx arrays, concatenated into [128, POS/16]
            wrapped = [
                _wrap_idx(idx_local[pass_pos_lo[r]:pass_pos_lo[r + 1]])
                for r in range(c.NR) if pass_pos_lo[r + 1] > pass_pos_lo[r]
            ]
            idx16 = np.concatenate(wrapped, axis=1) if wrapped else np.zeros((128, 0), np.int16)
            cores.append(dict(idx16=idx16, colidx=colv))
        # gather op list: per pass, ops of <= NIMAX positions
        ops = []        # (pass, pos_lo, ni)
        for r in range(c.NR):
            p0, p1 = int(pass_pos_lo[r]), int(pass_pos_lo[r + 1])
            while p0 < p1:
                ni = min(c.NIMAX, p1 - p0)
                ops.append((r, p0, ni))
                p0 += ni
        return dict(R=R, RSZ=RSZ, C=C, POS=POS, chunks=chunks, ops=ops,
                    cores=cores, seg_of=seg_of)

    conv1 = build_conv(src2, c.N)                # gather from x rows
    conv2 = build_conv(gslot[src2], c.W * S)     # gather from h1all rows
    assert conv2["R"] <= c.W * S

    # pooling plan
    gos_all = []
    for ci in range(c.W):
        gos = np.full(S, -1, np.int64)
        lo, hi = node_lo[ci], node_lo[ci + 1]
        gos[:hi - lo] = batch[lo:hi] - ci * c.GPC
        gos_all.append(gos)
    win_lo = np.zeros(nSC, np.int64)
    prev = 0
    for k in range(nSC):
        lo_k, hi_k = c.GPC, -1
        for gos in gos_all:
            seg = gos[k * 128:(k + 1) * 128]
            v = seg[seg >= 0]
            if len(v):
                lo_k = min(lo_k, int(v.min()))
                hi_k = max(hi_k, int(v.max()))
        if hi_k < 0:
            lo_k = hi_k = min(prev, c.GPC - 1)
        assert hi_k - lo_k + 1 <= c.GW, f"pool window too wide: {lo_k}..{hi_k}"
        lo_k = max(0, min(lo_k, c.GPC - c.GW))
        assert lo_k <= prev + c.GW, "pool window coverage gap"
        win_lo[k] = lo_k
        prev = max(prev, lo_k + c.GW - 1)
    covered = np.zeros(c.GPC, bool)
    for k in range(nSC):
        covered[win_lo[k]:win_lo[k] + c.GW] = True
    assert covered.all()

    pmats = []
    for ci in range(c.W):
        pmat = np.zeros((128, nSC * c.GW), np.float32)
        gos = gos_all[ci]
        for k in range(nSC):
            seg = gos[k * 128:(k + 1) * 128]
            for p in range(128):
                if seg[p] >= 0:
                    w = int(seg[p] - win_lo[k])
                    pmat[p, k * c.GW + w] = 1.0
        pmats.append(pmat)

    return dict(S=S, nbin=nbin, nSC=nSC, win_lo=win_lo, conv=[conv1, conv2],
                n_real=n_real, pmats=pmats)


# ---------------------------------------------------------------- program

def _build(plan, cfg):
    c = cfg
    S, nbin, nSC = plan["S"], plan["nbin"], plan["nSC"]
    win_lo = plan["win_lo"]
    D, OUT, FIN, GPC = c.D, c.OUT, c.FIN, c.GPC
    rg = [list(range(c.W))]
    nG = S // 512
    GDT = c.GDT

    nc = bacc.Bacc(num_devices=c.W)

    # ---- external inputs
    xg_d = nc.dram_tensor("xg", [plan["conv"][0]["RSZ"] * c.NR, D], GDT,
                          kind="ExternalInput")
    pmat_d = nc.dram_tensor("pmat", [128, nSC * c.GW], F32, kind="ExternalInput")
    idx_d, col_d = [], []
    for li in (0, 1):
        cv = plan["conv"][li]
        idx_d.append(nc.dram_tensor(f"idx{li}", [128, cv["POS"] // 16], I16,
                                    kind="ExternalInput"))
        col_d.append(nc.dram_tensor(f"col{li}", [128, cv["C"]], GDT,
                                    kind="ExternalInput"))
    code_d = nc.dram_tensor("code", [GPC, D], F32, kind="ExternalInput")
    ident_d = nc.dram_tensor("ident", [128, 128], F32, kind="ExternalInput")
    nh_d = nc.dram_tensor("nh", [128, 1], F32, kind="ExternalInput")

    wspec = {
        "c1_w1": [D, D], "c1_b1": [D], "c1_gamma": [D], "c1_beta": [D],
        "c1_w2": [D, D], "c1_b2": [D],
        "c2_w1": [D, D], "c2_b1": [D], "c2_gamma": [D], "c2_beta": [D],
        "c2_w2": [D, D], "c2_b2": [D],
        "g_l1_w": [D, D], "g_l1_b": [D], "g_l2_w": [D, OUT], "g_l2_b": [OUT],
        "fc1_w": [D, D], "fc1_b": [D], "fc2_w": [D, D], "fc2_b": [D],
        "fc3_w": [D, OUT], "fc3_b": [OUT],
        "fin_w": [2 * OUT, FIN], "fin_b": [FIN],
    }
    wd = {k: nc.dram_tensor(k, v, F32, kind="ExternalInput") for k, v in wspec.items()}

    out_d = nc.dram_tensor("out", [FIN, GPC], F32, kind="ExternalOutput")

    # ---- internal DRAM
    h1loc_d = nc.dram_tensor("h1loc", [S, D], GDT)
    RSZ2 = plan["conv"][1]["RSZ"]
    h1all_d = nc.dram_tensor("h1all", [RSZ2 * c.NR, D], GDT, addr_space="Shared")
    ar_in = [nc.dram_tensor(f"ar{i}i", [128, 2], F32) for i in (1, 2)]
    ar_out = [nc.dram_tensor(f"ar{i}o", [128, 2], F32, addr_space="Shared")
              for i in (1, 2)]

    with tile.TileContext(nc) as tc, ExitStack() as ctx:
        const = ctx.enter_context(tc.tile_pool(name="const", bufs=1))
        work = ctx.enter_context(tc.tile_pool(name="work", bufs=3))
        gwork = ctx.enter_context(tc.tile_pool(name="gwork", bufs=4))
        swork = ctx.enter_context(tc.tile_pool(name="swork", bufs=6))
        wide = ctx.enter_context(tc.tile_pool(name="wide", bufs=1))
        pp = ctx.enter_context(tc.tile_pool(name="pp", bufs=2, space="PSUM"))
        pp3 = ctx.enter_context(tc.tile_pool(name="pp3", bufs=3, space="PSUM"))

        def cload(dram_ap, shape, dtype, tag):
            t = const.tile(shape, dtype, tag=tag)
            nc.sync.dma_start(out=t[:], in_=dram_ap)
            return t

        ident_s = cload(ident_d[:], [128, 128], F32, "ident")
        nh_s = cload(nh_d[:], [128, 1], F32, "nh")
        pmat_s = cload(pmat_d[:], [128, nSC * c.GW], F32, "pmat")

        ws = {}
        for k, shp in wspec.items():
            if len(shp) == 2:
                ws[k] = cload(wd[k][:], shp, F32, k)
            else:
                ws[k] = cload(wd[k][:, None], [shp[0], 1], F32, k)
        finw_hi = const.tile([OUT, FIN], F32, tag="finw_hi")
        nc.sync.dma_start(out=finw_hi[:], in_=wd["fin_w"][OUT:2 * OUT, :])

        # iota row pattern repeated (for one-hot gen), in gather dtype
        IOB = 8  # chunks per one-hot op
        iota_i = const.tile([128, IOB * 128], I32, tag="iota_i")
        nc.gpsimd.iota(iota_i[:], pattern=[[0, IOB], [1, 128]], base=0,
                       channel_multiplier=0)
        iota_s = const.tile([128, IOB * 128], GDT, tag="iota_s")
        nc.vector.tensor_copy(out=iota_s[:], in_=iota_i[:])

        ones_d1 = const.tile([OUT, 1], F32, tag="ones_d1")
        nc.vector.memset(ones_d1[:], 1.0)
        ones_1d = const.tile([1, OUT], F32, tag="ones_1d")
        nc.vector.memset(ones_1d[:], 1.0)
        ones_f1 = const.tile([FIN, 1], F32, tag="ones_f1")
        nc.vector.memset(ones_f1[:], 1.0)
        ones_1f = const.tile([1, FIN], F32, tag="ones_1f")
        nc.vector.memset(ones_1f[:], 1.0)

        # =========================== code MLP branch (fills bubbles)
        nbl = (GPC + 127) // 128
        code_nm = const.tile([128, nbl * D], F32, tag="code_nm")
        nc.sync.dma_start(
            out=code_nm[:].rearrange("p (b f) -> p b f", b=nbl),
            in_=code_d[:].rearrange("(b p) f -> p b f", p=128))
        codeT = const.tile([128, GPC], F32, tag="codeT")
        for b in range(nbl):
            tp = pp.tile([128, 128], F32, tag="tp")
            nc.tensor.transpose(out=tp[:], in_=code_nm[:, b * D:(b + 1) * D],
                                identity=ident_s[:])
            nc.vector.tensor_copy(out=codeT[:, b * 128:(b + 1) * 128], in_=tp[:])
        cps = pp3.tile([128, GPC], F32, tag="zp")
        nc.tensor.matmul(out=cps[:], lhsT=ws["fc1_w"][:], rhs=codeT[:],
                         start=True, stop=True)
        c1_s = const.tile([128, GPC], F32, tag="c1_s")
        nc.scalar.activation(out=c1_s[:], in_=cps[:], func=AF.Relu,
                             bias=ws["fc1_b"][:, :1])
        cps2 = pp3.tile([128, GPC], F32, tag="zp")
        nc.tensor.matmul(out=cps2[:], lhsT=ws["fc2_w"][:], rhs=c1_s[:],
                         start=True, stop=True)
        c2_s = const.tile([128, GPC], F32, tag="c2_s")
        nc.scalar.activation(out=c2_s[:], in_=cps2[:], func=AF.Relu,
                             bias=ws["fc2_b"][:, :1])
        cps3 = pp.tile([OUT, GPC], F32, tag="up")
        nc.tensor.matmul(out=cps3[:], lhsT=ws["fc3_w"][:], rhs=c2_s[:],
                         start=True, stop=True)
        c3_s = const.tile([OUT, GPC], F32, tag="c3_s")
        nc.scalar.activation(out=c3_s[:], in_=cps3[:], func=AF.Identity,
                             bias=ws["fc3_b"][:, :1])
        e64 = const.tile([OUT, GPC], F32, tag="e64")
        nc.scalar.activation(out=e64[:], in_=c3_s[:], func=AF.Exp)
        lsp = pp.tile([1, GPC], F32, tag="tp")
        nc.tensor.matmul(out=lsp[:], lhsT=ones_d1[:], rhs=e64[:],
                         start=True, stop=True)
        lse_s = const.tile([1, GPC], F32, tag="lse_s")
        nc.scalar.activation(out=lse_s[:], in_=lsp[:], func=AF.Ln)
        bcp = pp.tile([OUT, GPC], F32, tag="up")
        nc.tensor.matmul(out=bcp[:], lhsT=ones_1d[:], rhs=lse_s[:],
                         start=True, stop=True)
        code_embT = const.tile([OUT, GPC], F32, tag="code_embT")
        nc.vector.tensor_tensor(out=code_embT[:], in0=c3_s[:], in1=bcp[:],
                                op=ALU.subtract)

        # =========================== GIN convs
        zu_t = wide.tile([128, S], F32, tag="zu")     # z, then u, then zb (in place)
        pooled_acc = const.tile([128, GPC], F32, tag="pooled_acc")
        nc.vector.memset(pooled_acc[:], 0.0)
        ag_inst = None

        idxcol = {}
        for li, cv_ in enumerate(plan["conv"]):
            i_s = const.tile([128, cv_["POS"] // 16], I16, tag=f"idx{li+1}")
            nc.sync.dma_start(out=i_s[:], in_=idx_d[li][:])
            c_s = const.tile([128, cv_["C"]], GDT, tag=f"col{li+1}")
            nc.sync.dma_start(out=c_s[:], in_=col_d[li][:])
            idxcol[li + 1] = (i_s, c_s)

        def conv(idx, cv, src_dram, idx_dram, col_dram,
                 w1_s, b1_s, gam_s, bet_s, w2_s, b2_s, ari, aro, dep=None,
                 upto="full"):
            C, POS = cv["C"], cv["POS"]
            chunks, ops = cv["chunks"], cv["ops"]
            idx_s, col_s = idxcol[idx]
            ssum = const.tile([128, nG], F32, tag=f"ssum{idx}")
            ssq = const.tile([128, nG], F32, tag=f"ssq{idx}")

            # map chunk -> (op index, block within op)
            chunk_op = []
            for oi, (r, plo, ni) in enumerate(ops):
                for b in range(ni // 128):
                    chunk_op.append((oi, b))
            assert len(chunk_op) == C

            gtiles = {}
            stiles = {}
            cur_group = None       # (bin, pass)
            zp = None
            group_started = set()  # bins with first (copy) group done

            def close_group():
                nonlocal cur_group, zp
                if cur_group is None:
                    return
                bn = cur_group[0]
                cols = slice(bn * 512, (bn + 1) * 512)
                if bn in group_started:
                    nc.vector.tensor_tensor(out=zu_t[:, cols], in0=zu_t[:, cols],
                                            in1=zp[:], op=ALU.add)
                else:
                    nc.vector.tensor_copy(out=zu_t[:, cols], in_=zp[:])
                    group_started.add(bn)
                cur_group, zp = None, None

            for ci in range(C):
                r, sb = chunks[ci]
                bn, sl4 = sb // 4, sb % 4
                oi, blk = chunk_op[ci]
                if oi not in gtiles:
                    opr, plo, ni = ops[oi]
                    gt = gwork.tile([128, c.NIMAX], GDT, tag="gt")
                    src_view = src_dram[:].rearrange(
                        "(q four) f -> four q f", four=c.NR)[opr]
                    g_ins = nc.gpsimd.dma_gather(
                        gt[:, :ni].rearrange("p (k f) -> p k f", k=ni // 128),
                        src_view,
                        idx_s[:, plo // 16:(plo + ni) // 16],
                        ni, ni, 128, elem_step=c.NR * D,
                        single_packet=False)
                    if dep is not None:
                        add_dep_helper(g_ins.ins, dep.ins, True, "gather after AG")
                    gtiles = {oi: gt}
                if ci % IOB == 0:
                    nob = min(IOB, C - ci)
                    st = swork.tile([128, IOB * 128], GDT, tag="st")
                    nc.vector.tensor_tensor(
                        out=st[:, :nob * 128].rearrange("p (c f) -> p c f", c=nob),
                        in0=col_s[:, ci:ci + nob].to_broadcast([128, nob, 128]),
                        in1=iota_s[:, :nob * 128].rearrange("p (c f) -> p c f", c=nob),
                        op=ALU.is_equal)
                    stiles = {ci // IOB: st}
                if cur_group != (bn, r):
                    close_group()
                    cur_group = (bn, r)
                    zp = pp3.tile([128, 512], F32, tag="zp")
                # start flag: first chunk of this (bin, pass) group
                is_first = (ci == 0 or chunks[ci - 1][0] != r
                            or chunks[ci - 1][1] // 4 != bn)
                is_last = (ci == C - 1 or chunks[ci + 1][0] != chunks[ci][0]
                           or chunks[ci + 1][1] // 4 != bn)
                nc.tensor.matmul(
                    out=zp[:, sl4 * 128:(sl4 + 1) * 128],
                    lhsT=gtiles[oi][:, blk * 128:(blk + 1) * 128],
                    rhs=stiles[ci // IOB][:, (ci % IOB) * 128:(ci % IOB + 1) * 128],
                    start=is_first, stop=is_last,
                    skip_group_check=True)
            close_group()
            if upto == "agg":
                return

            # ---- layer 1 + stats
            for g in range(nG):
                cols = slice(g * 512, (g + 1) * 512)
                up = pp.tile([128, 512], F32, tag="up")
                nc.tensor.matmul(out=up[:], lhsT=w1_s[:], rhs=zu_t[:, cols],
                                 start=True, stop=True)
                nc.scalar.activation(out=zu_t[:, cols], in_=up[:],
                                     func=AF.Identity, bias=b1_s[:, :1],
                                     accum_out=ssum[:, g:g + 1])
                sq = work.tile([128, 512], F32, tag="sq")
                nc.scalar.activation(out=sq[:], in_=zu_t[:, cols],
                                     func=AF.Square,
                                     accum_out=ssq[:, g:g + 1])

            # ---- BN stats + AllReduce
            sum_r = const.tile([128, 1], F32, tag=f"sum_r{idx}")
            ssq_r = const.tile([128, 1], F32, tag=f"ssq_r{idx}")
            nc.vector.tensor_reduce(out=sum_r[:], in_=ssum[:],
                                    axis=mybir.AxisListType.X, op=ALU.add)
            nc.vector.tensor_reduce(out=ssq_r[:], in_=ssq[:],
                                    axis=mybir.AxisListType.X, op=ALU.add)
            b1sq = const.tile([128, 1], F32, tag=f"b1sq{idx}")
            nc.scalar.activation(out=b1sq[:], in_=b1_s[:], func=AF.Square)
            tmp1 = const.tile([128, 1], F32, tag=f"tmp1_{idx}")
            nc.vector.tensor_tensor(out=tmp1[:], in0=b1_s[:], in1=nh_s[:],
                                    op=ALU.mult)
            nc.vector.tensor_tensor(out=sum_r[:], in0=sum_r[:], in1=tmp1[:],
                                    op=ALU.subtract)
            nc.vector.tensor_tensor(out=tmp1[:], in0=b1sq[:], in1=nh_s[:],
                                    op=ALU.mult)
            nc.vector.tensor_tensor(out=ssq_r[:], in0=ssq_r[:], in1=tmp1[:],
                                    op=ALU.subtract)
            if upto == "stats":
                return
            pack = const.tile([128, 2], F32, tag=f"pack{idx}")
            nc.vector.tensor_copy(out=pack[:, 0:1], in_=sum_r[:])
            nc.vector.tensor_copy(out=pack[:, 1:2], in_=ssq_r[:])
            nc.sync.dma_start(out=ari[:], in_=pack[:])
            ar = nc.gpsimd.collective_compute(
                "AllReduce", ALU.add, replica_groups=rg,
                ins=[ari[:]], outs=[aro[:]])
            rb = const.tile([128, 2], F32, tag=f"rb{idx}")
            d = nc.sync.dma_start(out=rb[:], in_=aro[:])
            add_dep_helper(d.ins, ar.ins, True, "read after AR")
            mean = const.tile([128, 1], F32, tag=f"mean{idx}")
            m2 = const.tile([128, 1], F32, tag=f"m2{idx}")
            nc.scalar.activation(out=mean[:], in_=rb[:, 0:1], func=AF.Copy,
                                 scale=1.0 / c.N)
            nc.scalar.activation(out=m2[:], in_=rb[:, 1:2], func=AF.Copy,
                                 scale=1.0 / c.N)
            msq = const.tile([128, 1], F32, tag=f"msq{idx}")
            nc.scalar.activation(out=msq[:], in_=mean[:], func=AF.Square)
            var = const.tile([128, 1], F32, tag=f"var{idx}")
            nc.vector.tensor_tensor(out=var[:], in0=m2[:], in1=msq[:],
                                    op=ALU.subtract)
            nc.vector.tensor_scalar_add(out=var[:], in0=var[:], scalar1=BN_EPS)
            std = const.tile([128, 1], F32, tag=f"std{idx}")
            nc.scalar.activation(out=std[:], in_=var[:], func=AF.Sqrt)
            inv = const.tile([128, 1], F32, tag=f"inv{idx}")
            nc.vector.reciprocal(out=inv[:], in_=std[:])
            sc = const.tile([128, 1], F32, tag=f"sc{idx}")
            nc.vector.tensor_tensor(out=sc[:], in0=gam_s[:], in1=inv[:],
                                    op=ALU.mult)
            sh = const.tile([128, 1], F32, tag=f"sh{idx}")
            nc.vector.tensor_tensor(out=sh[:], in0=mean[:], in1=sc[:],
                                    op=ALU.mult)
            nc.vector.tensor_tensor(out=sh[:], in0=bet_s[:], in1=sh[:],
                                    op=ALU.subtract)
            if upto == "bn":
                return

            # ---- BN apply + relu (in place), layer 2, transposes
            for g in range(nG):
                cols = slice(g * 512, (g + 1) * 512)
                nc.scalar.activation(out=zu_t[:, cols], in_=zu_t[:, cols],
                                     func=AF.Relu, bias=sh[:, :1],
                                     scale=sc[:, :1])
                hp = pp.tile([128, 512], F32, tag="up")
                nc.tensor.matmul(out=hp[:], lhsT=w2_s[:], rhs=zu_t[:, cols],
                                 start=True, stop=True)
                hb = work.tile([128, 512], F32, tag="hb")
                nc.scalar.activation(out=hb[:], in_=hp[:], func=AF.Relu,
                                     bias=b2_s[:, :1])
                hnm = work.tile([128, 4 * D], GDT if idx == 1 else F32, tag="hnm")
                for t in range(4):
                    tp = pp.tile([128, 128], F32, tag="tp")
                    nc.tensor.transpose(out=tp[:], in_=hb[:, t * 128:(t + 1) * 128],
                                        identity=ident_s[:])
                    nc.vector.tensor_copy(out=hnm[:, t * D:(t + 1) * D], in_=tp[:])
                    if idx == 2:
                        k = g * 4 + t
                        lo = int(win_lo[k])
                        poolw = pp.tile([128, c.GW], F32, tag="tp")
                        nc.tensor.matmul(
                            out=poolw[:],
                            lhsT=hnm[:, t * D:(t + 1) * D],
                            rhs=pmat_s[:, k * c.GW:(k + 1) * c.GW],
                            start=True, stop=True)
                        nc.vector.tensor_tensor(
                            out=pooled_acc[:, lo:lo + c.GW],
                            in0=pooled_acc[:, lo:lo + c.GW],
                            in1=poolw[:], op=ALU.add)
                if idx == 1:
                    nc.sync.dma_start(
                        out=h1loc_d[g * 512:(g + 1) * 512, :].rearrange(
                            "(b p) f -> p b f", p=128),
                        in_=hnm[:].rearrange("p (b f) -> p b f", b=4))

        cvs = plan["conv"]
        dbg = c.DBG
        upto1 = {1: "agg", 2: "stats", 3: "bn"}.get(dbg, "full")
        conv(1, cvs[0], xg_d, idx_d[0], col_d[0],
             ws["c1_w1"], ws["c1_b1"], ws["c1_gamma"], ws["c1_beta"],
             ws["c1_w2"], ws["c1_b2"], ar_in[0], ar_out[0], upto=upto1)
        if dbg >= 5:
            ag_inst = nc.gpsimd.collective_compute(
                "AllGather", ALU.bypass, replica_groups=rg,
                ins=[h1loc_d[:]], outs=[h1all_d[:]])
        if dbg >= 6:
            # conv2 gathers must run after the AllGather lands
            conv(2, cvs[1], h1all_d, idx_d[1], col_d[1],
                 ws["c2_w1"], ws["c2_b1"], ws["c2_gamma"], ws["c2_beta"],
                 ws["c2_w2"], ws["c2_b2"], ar_in[1], ar_out[1], dep=ag_inst)
        if dbg < 99:
            pout = const.tile([FIN, GPC], F32, tag="outT")
            nc.vector.tensor_copy(out=pout[:], in_=zu_t[0:FIN, 0:GPC])
            nc.sync.dma_start(out=out_d[:], in_=pout[:])
        else:
            # =========================== head
            hd1 = pp3.tile([128, GPC], F32, tag="zp")
            nc.tensor.matmul(out=hd1[:], lhsT=ws["g_l1_w"][:], rhs=pooled_acc[:],
                             start=True, stop=True)
            t_s = const.tile([128, GPC], F32, tag="t_s")
            nc.scalar.activation(out=t_s[:], in_=hd1[:], func=AF.Relu,
                                 bias=ws["g_l1_b"][:, :1])
            hd2 = pp.tile([OUT, GPC], F32, tag="up")
            nc.tensor.matmul(out=hd2[:], lhsT=ws["g_l2_w"][:], rhs=t_s[:],
                             start=True, stop=True)
            trans_embT = const.tile([OUT, GPC], F32, tag="trans_embT")
            nc.scalar.activation(out=trans_embT[:], in_=hd2[:], func=AF.Identity,
                                 bias=ws["g_l2_b"][:, :1])
            fp = pp.tile([FIN, GPC], F32, tag="tp")
            nc.tensor.matmul(out=fp[:], lhsT=ws["fin_w"][0:OUT, :],
                             rhs=code_embT[:], start=True, stop=False,
                             skip_group_check=True)
            nc.tensor.matmul(out=fp[:], lhsT=finw_hi[:],
                             rhs=trans_embT[:], start=False, stop=True,
                             skip_group_check=True)
            f_s = const.tile([FIN, GPC], F32, tag="f_s")
            nc.scalar.activation(out=f_s[:], in_=fp[:], func=AF.Identity,
                                 bias=ws["fin_b"][:, :1])
            ef = const.tile([FIN, GPC], F32, tag="ef")
            nc.scalar.activation(out=ef[:], in_=f_s[:], func=AF.Exp)
            lfp = pp.tile([1, GPC], F32, tag="up")
            nc.tensor.matmul(out=lfp[:], lhsT=ones_f1[:], rhs=ef[:],
                             start=True, stop=True)
            lf_s = const.tile([1, GPC], F32, tag="lf_s")
            nc.scalar.activation(out=lf_s[:], in_=lfp[:], func=AF.Ln)
            bfp = pp3.tile([FIN, GPC], F32, tag="zp")
            nc.tensor.matmul(out=bfp[:], lhsT=ones_1f[:], rhs=lf_s[:],
                             start=True, stop=True)
            outT = const.tile([FIN, GPC], F32, tag="outT")
            nc.vector.tensor_tensor(out=outT[:], in0=f_s[:], in1=bfp[:],
                                    op=ALU.subtract)
            nc.sync.dma_start(out=out_d[:], in_=outT[:])

    # order conv2 gathers after the AllGather
    if not nc.is_finalized():
        nc.finalize()
    return nc


# ---------------------------------------------------------------- runner

def make_in_maps(inputs, plan, cfg):
    c = cfg
    wnames = ["c1_w1", "c1_b1", "c1_gamma", "c1_beta", "c1_w2", "c1_b2",
              "c2_w1", "c2_b1", "c2_gamma", "c2_beta", "c2_w2", "c2_b2",
              "g_l1_w", "g_l1_b", "g_l2_w", "g_l2_b",
              "fc1_w", "fc1_b", "fc2_w", "fc2_b", "fc3_w", "fc3_b",
              "fin_w", "fin_b"]
    np_gdt = np.float32 if c.GDT == F32 else __import__("ml_dtypes").bfloat16
    x = np.asarray(inputs["x"], np.float32)
    R1, RSZ1 = plan["conv"][0]["R"], plan["conv"][0]["RSZ"]
    xg = np.zeros((RSZ1 * c.NR, c.D), np_gdt)
    xg[:x.shape[0]] = x.astype(np_gdt)
    code = np.ascontiguousarray(np.asarray(inputs["code_x"], np.float32))
    ident = np.eye(128, dtype=np.float32)
    in_maps = []
    for ci in range(c.W):
        m = {
            "xg": xg,
            "pmat": plan["pmats"][ci],
            "code": code[ci * c.GPC:(ci + 1) * c.GPC],
            "ident": ident,
            "nh": np.full((128, 1), float(plan["S"] - plan["n_real"][ci]),
                          np.float32),
        }
        for li in (0, 1):
            cv = plan["conv"][li]
            m[f"idx{li}"] = cv["cores"][ci]["idx16"]
            m[f"col{li}"] = cv["cores"][ci]["colidx"].astype(np_gdt)
        for k in wnames:
            m[k] = np.ascontiguousarray(np.asarray(inputs[k], np.float32))
        in_maps.append(m)
    return in_maps


_CACHE = {}


def _get_compiled(inputs, cfg):
    if "prog" not in _CACHE:
        plan = _plan(inputs["edge_index"], inputs["batch"], cfg)
        nc = _build(plan, cfg)
        _CACHE["prog"] = (plan, nc)
    return _CACHE["prog"]


def kernel(**inputs) -> np.ndarray:
    cfg = DEFAULT_CFG
    plan, nc = _get_compiled(inputs, cfg)
    in_maps = make_in_maps(inputs, plan, cfg)
    res = run_bass_kernel_spmd(nc, in_maps, core_ids=list(range(cfg.W)))
    outs = [res.results[ci]["out"].T for ci in range(cfg.W)]
    return np.ascontiguousarray(np.concatenate(outs, axis=0).astype(np.float32))



# revision 2
# speedup vs baseline: 1.1406x; 1.1406x over previous
"""Trainium2 Bass kernel for the GIN message-passing model (8 NeuronCores).

Sharding: graph partitioning.  Core c owns graphs [c*G/8, (c+1)*G/8) and the
contiguous node range of those graphs (batch is sorted), plus every edge whose
dst lands there.  dst nodes get compact slot ranks.

Aggregation: edges gathered with `dma_gather` (int16 indices => source row
space split into 4 contiguous ranges = 4 passes).  Gathers cycle across 4
SWDGE queues (ops of <=2048 idxs) so SDMA descriptor drains overlap ~4x.
Within a pass edges are dst-sorted and packed into 128-position chunks
aligned to 128-slot subbins; a chunk's segment-sum is one matmul (gathered
rows stationary, on-chip one-hot moving) into the subbin's slice of a
512-slot PSUM bin.  Self-edges are NOT gathered: z is pre-initialized with
the node's own features (host-transposed x for conv1; in-place h1 for conv2).

conv layer-1 matmuls are interleaved per-bin into the chunk stream so they
overlap the gather tail.  conv2's 4 source ranges are slot-quarters of each
core's h1, published by 4 pipelined AllGathers so conv2 pass-q gathers start
as soon as AG_q lands.  conv2's layer-2 runs node-major (no transposes),
feeding pooling matmuls directly; BN pad corrections are computed on device.
"""

import sys

for _p in ("/opt/trn_rl_repo",):
    if _p not in sys.path:
        sys.path.insert(0, _p)

import numpy as np
from contextlib import ExitStack

import concourse.bass as bass
import concourse.bacc as bacc
import concourse.mybir as mybir
import concourse.tile as tile
from concourse.bass_utils import run_bass_kernel_spmd
from concourse.tile_rust import add_dep_helper

F32 = mybir.dt.float32
BF16 = mybir.dt.bfloat16
I32 = mybir.dt.int32
I16 = mybir.dt.int16
AF = mybir.ActivationFunctionType
ALU = mybir.AluOpType

BN_EPS = 1e-5
PADCOL = 200.0          # colidx value for pad positions (never matches 0..127)


class Cfg:
    def __init__(self, N=100000, E=500000, G=2048, D=128, OUT=64, FIN=2, W=8,
                 NR=4, NIMAX=2048, NQ=4, GW=32, GDT=BF16):
        self.N, self.E, self.G, self.D, self.OUT, self.FIN, self.W = N, E, G, D, OUT, FIN, W
        self.NR = NR        # source ranges (int16 index limit)
        self.NIMAX = NIMAX  # max positions per dma_gather
        self.NQ = NQ        # SWDGE queues to cycle gathers over
        self.GW = GW        # pooling window width (graphs)
        self.GDT = GDT      # gather dtype (bf16 or f32)
        self.GPC = G // W   # graphs per core


DEFAULT_CFG = Cfg()


def _wrap_idx(lst):
    """dma_gather index layout: position j is read from row j%16, col j//16."""
    assert len(lst) % 16 == 0
    return np.tile(np.asarray(lst, np.int16).reshape(-1, 16).T, (8, 1))


# ---------------------------------------------------------------- host plan

def _plan(edge_index, batch, cfg):
    c = cfg
    batch = np.asarray(batch).astype(np.int64)
    ei = np.asarray(edge_index).astype(np.int64)
    owner = (batch // c.GPC).astype(np.int64)

    src, dst = ei[0], ei[1]          # no synthetic self-edges
    eowner = owner[dst]

    # compact slot ranks per core
    n_real = np.zeros(c.W, np.int64)
    slot_of = np.full(c.N, -1, np.int64)
    node_lo = np.zeros(c.W + 1, np.int64)
    for ci in range(c.W):
        node_lo[ci] = np.searchsorted(batch, ci * c.GPC)
    node_lo[c.W] = c.N
    for ci in range(c.W):
        lo, hi = node_lo[ci], node_lo[ci + 1]
        n_real[ci] = hi - lo
        slot_of[lo:hi] = np.arange(hi - lo)
    S = int(((n_real.max() + 511) // 512) * 512)
    nbin = S // 512
    nsub = S // 128
    nSC = S // 128

    # conv1 source ranges: contiguous quarters of the x row space
    r1_lo = np.array([(c.N * q) // c.NR for q in range(c.NR + 1)], np.int64)
    assert (np.diff(r1_lo) <= 32767).all()
    epass1 = np.searchsorted(r1_lo[1:], src, side="right")
    loc1 = src - r1_lo[epass1]

    # conv2 source ranges: slot-quarters (bin-aligned groups of each core's S)
    base, rem = nbin // c.NR, nbin % c.NR
    nbq = [base + (1 if q < rem else 0) for q in range(c.NR)]
    qslot_lo = np.zeros(c.NR + 1, np.int64)
    for q in range(c.NR):
        qslot_lo[q + 1] = qslot_lo[q] + nbq[q] * 512
    Qsz = np.diff(qslot_lo)
    assert (c.W * Qsz <= 32767).all()
    s_src = slot_of[src]
    o_src = owner[src]
    epass2 = np.searchsorted(qslot_lo[1:], s_src, side="right")
    loc2 = o_src * Qsz[epass2] + (s_src - qslot_lo[epass2])

    def build_conv(epass, loc):
        """epass: per-edge source range id; loc: row within that range.
        Returns common chunk structure + per-core idx/colidx arrays."""
        counts = np.zeros((c.W, c.NR, nsub), np.int64)
        percore_edges = []
        for ci in range(c.W):
            m = eowner == ci
            sl = slot_of[dst[m]]
            pr = epass[m]
            rows = loc[m]
            sub = sl // 128
            order = np.lexsort((sl, sub, pr))
            sl, pr, rows, sub = sl[order], pr[order], rows[order], sub[order]
            np.add.at(counts[ci], (pr, sub), 1)
            percore_edges.append((sl, pr, rows, sub))
        # common chunk structure (max over cores per segment)
        nch = -(-counts.max(axis=0) // 128)               # [NR, nsub]
        chunks = []     # (pass, subbin)
        for r in range(c.NR):
            for sb in range(nsub):
                for k in range(int(nch[r, sb])):
                    chunks.append((r, sb))
        C = len(chunks)
        POS = C * 128
        pass_pos_lo = np.zeros(c.NR + 1, np.int64)
        for r in range(c.NR):
            pass_pos_lo[r + 1] = pass_pos_lo[r] + 128 * int(nch[r].sum())
        # per-core arrays
        cores = []
        for ci in range(c.W):
            sl, pr, rows, sub = percore_edges[ci]
            idx_local = np.zeros(POS, np.int64)          # pad -> row 0 of range
            colv = np.full((128, C), PADCOL, np.float64)
            seg_base = {}
            pos = 0
            for r in range(c.NR):
                for sb in range(nsub):
                    seg_base[(r, sb)] = pos
                    pos += 128 * int(nch[r, sb])
            key = pr * nsub + sub
            uniq, start_idx = np.unique(key, return_index=True)
            end_idx = np.append(start_idx[1:], len(key))
            for u, s0, s1 in zip(uniq, start_idx, end_idx):
                r, sb = int(u) // nsub, int(u) % nsub
                base_ = seg_base[(r, sb)]
                n = s1 - s0
                p = base_ + np.arange(n)
                idx_local[p] = rows[s0:s1]
                colv[p % 128, p // 128] = sl[s0:s1] - sb * 128
            wrapped = [
                _wrap_idx(idx_local[pass_pos_lo[r]:pass_pos_lo[r + 1]])
                for r in range(c.NR) if pass_pos_lo[r + 1] > pass_pos_lo[r]
            ]
            idx16 = np.concatenate(wrapped, axis=1) if wrapped else np.zeros((128, 0), np.int16)
            cores.append(dict(idx16=idx16, colidx=colv))
        # gather op list: per pass, ops of <= NIMAX positions
        ops = []        # (pass, pos_lo, ni)
        for r in range(c.NR):
            p0, p1 = int(pass_pos_lo[r]), int(pass_pos_lo[r + 1])
            while p0 < p1:
                ni = min(c.NIMAX, p1 - p0)
                ops.append((r, p0, ni))
                p0 += ni
        return dict(C=C, POS=POS, chunks=chunks, ops=ops, cores=cores)

    conv1 = build_conv(epass1, loc1)
    conv2 = build_conv(epass2, loc2)

    # pooling plan
    gos_all = []
    for ci in range(c.W):
        gos = np.full(S, -1, np.int64)
        lo, hi = node_lo[ci], node_lo[ci + 1]
        gos[:hi - lo] = batch[lo:hi] - ci * c.GPC
        gos_all.append(gos)
    win_lo = np.zeros(nSC, np.int64)
    prev = 0
    for k in range(nSC):
        lo_k, hi_k = c.GPC, -1
        for gos in gos_all:
            seg = gos[k * 128:(k + 1) * 128]
            v = seg[seg >= 0]
            if len(v):
                lo_k = min(lo_k, int(v.min()))
                hi_k = max(hi_k, int(v.max()))
        if hi_k < 0:
            lo_k = hi_k = min(prev, c.GPC - 1)
        assert hi_k - lo_k + 1 <= c.GW, f"pool window too wide: {lo_k}..{hi_k}"
        lo_k = max(0, min(lo_k, c.GPC - c.GW))
        assert lo_k <= prev + c.GW, "pool window coverage gap"
        win_lo[k] = lo_k
        prev = max(prev, lo_k + c.GW - 1)
    covered = np.zeros(c.GPC, bool)
    for k in range(nSC):
        covered[win_lo[k]:win_lo[k] + c.GW] = True
    assert covered.all()

    pmats = []
    for ci in range(c.W):
        pmat = np.zeros((128, nSC * c.GW), np.float32)
        gos = gos_all[ci]
        for k in range(nSC):
            seg = gos[k * 128:(k + 1) * 128]
            for p in range(128):
                if seg[p] >= 0:
                    w = int(seg[p] - win_lo[k])
                    pmat[p, k * c.GW + w] = 1.0
        pmats.append(pmat)

    return dict(S=S, nbin=nbin, nSC=nSC, win_lo=win_lo, conv=[conv1, conv2],
                n_real=n_real, node_lo=node_lo, r1_lo=r1_lo,
                qslot_lo=qslot_lo, Qsz=Qsz, nbq=nbq, pmats=pmats)


# ---------------------------------------------------------------- program

def _build(plan, cfg):
    c = cfg
    S, nbin, nSC = plan["S"], plan["nbin"], plan["nSC"]
    win_lo = plan["win_lo"]
    qslot_lo, Qsz, nbq = plan["qslot_lo"], plan["Qsz"], plan["nbq"]
    r1_lo = plan["r1_lo"]
    D, OUT, FIN, GPC = c.D, c.OUT, c.FIN, c.GPC
    rg = [list(range(c.W))]
    nG = nbin
    GDT = c.GDT

    nc = bacc.Bacc(num_devices=c.W, num_swdge_queues=c.NQ)

    # ---- external inputs
    xg_d = nc.dram_tensor("xg", [c.N, D], GDT, kind="ExternalInput")
    xownT_d = nc.dram_tensor("xownT", [128, S], F32, kind="ExternalInput")
    pmat_d = nc.dram_tensor("pmat", [128, nSC * c.GW], F32, kind="ExternalInput")
    idx_d, col_d = [], []
    for li in (0, 1):
        cv = plan["conv"][li]
        idx_d.append(nc.dram_tensor(f"idx{li}", [128, cv["POS"] // 16], I16,
                                    kind="ExternalInput"))
        col_d.append(nc.dram_tensor(f"col{li}", [128, cv["C"]], GDT,
                                    kind="ExternalInput"))
    code_d = nc.dram_tensor("code", [GPC, D], F32, kind="ExternalInput")
    ident_d = nc.dram_tensor("ident", [128, 128], F32, kind="ExternalInput")
    nh_d = nc.dram_tensor("nh", [128, 1], F32, kind="ExternalInput")

    wspec = {
        "c1_w1": [D, D], "c1_b1": [D], "c1_gamma": [D], "c1_beta": [D],
        "c1_w2": [D, D], "c1_b2": [D],
        "c2_w1": [D, D], "c2_b1": [D], "c2_gamma": [D], "c2_beta": [D],
        "c2_w2": [D, D], "c2_b2": [D],
        "g_l1_w": [D, D], "g_l1_b": [D], "g_l2_w": [D, OUT], "g_l2_b": [OUT],
        "fc1_w": [D, D], "fc1_b": [D], "fc2_w": [D, D], "fc2_b": [D],
        "fc3_w": [D, OUT], "fc3_b": [OUT],
        "fin_w": [2 * OUT, FIN], "fin_b": [FIN],
    }
    wd = {k: nc.dram_tensor(k, v, F32, kind="ExternalInput") for k, v in wspec.items()}
    b2r_d = nc.dram_tensor("c2_b2r", [1, D], F32, kind="ExternalInput")

    out_d = nc.dram_tensor("out", [FIN, GPC], F32, kind="ExternalOutput")

    # ---- internal DRAM
    h1loc_d = nc.dram_tensor("h1loc", [S, D], GDT)
    h1all_d = [nc.dram_tensor(f"h1all{q}", [c.W * int(Qsz[q]), D], GDT,
                              addr_space="Shared") for q in range(c.NR)]
    ar_in = [nc.dram_tensor(f"ar{i}i", [128, 2], F32) for i in (1, 2)]
    ar_out = [nc.dram_tensor(f"ar{i}o", [128, 2], F32, addr_space="Shared")
              for i in (1, 2)]

    with tile.TileContext(nc) as tc, ExitStack() as ctx:
        const = ctx.enter_context(tc.tile_pool(name="const", bufs=1))
        work = ctx.enter_context(tc.tile_pool(name="work", bufs=3))
        gwork = ctx.enter_context(tc.tile_pool(name="gwork", bufs=8))
        swork = ctx.enter_context(tc.tile_pool(name="swork", bufs=6))
        wide = ctx.enter_context(tc.tile_pool(name="wide", bufs=1))
        pp = ctx.enter_context(tc.tile_pool(name="pp", bufs=2, space="PSUM"))
        pp3 = ctx.enter_context(tc.tile_pool(name="pp3", bufs=3, space="PSUM"))

        def cload(dram_ap, shape, dtype, tag):
            t = const.tile(shape, dtype, tag=tag)
            nc.sync.dma_start(out=t[:], in_=dram_ap)
            return t

        ident_s = cload(ident_d[:], [128, 128], F32, "ident")
        nh_s = cload(nh_d[:], [128, 1], F32, "nh")
        pmat_s = cload(pmat_d[:], [128, nSC * c.GW], F32, "pmat")
        b2r_s = cload(b2r_d[:], [1, D], F32, "b2r")

        ws = {}
        for k, shp in wspec.items():
            if len(shp) == 2:
                ws[k] = cload(wd[k][:], shp, F32, k)
            else:
                ws[k] = cload(wd[k][:, None], [shp[0], 1], F32, k)
        finw_hi = const.tile([OUT, FIN], F32, tag="finw_hi")
        nc.sync.dma_start(out=finw_hi[:], in_=wd["fin_w"][OUT:2 * OUT, :])

        # iota row pattern repeated (for one-hot gen), in gather dtype
        IOB = 8  # chunks per one-hot op
        iota_i = const.tile([128, IOB * 128], I32, tag="iota_i")
        nc.gpsimd.iota(iota_i[:], pattern=[[0, IOB], [1, 128]], base=0,
                       channel_multiplier=0)
        iota_s = const.tile([128, IOB * 128], GDT, tag="iota_s")
        nc.vector.tensor_copy(out=iota_s[:], in_=iota_i[:])

        ones_d1 = const.tile([OUT, 1], F32, tag="ones_d1")
        nc.vector.memset(ones_d1[:], 1.0)
        ones_1d = const.tile([1, OUT], F32, tag="ones_1d")
        nc.vector.memset(ones_1d[:], 1.0)
        ones_f1 = const.tile([FIN, 1], F32, tag="ones_f1")
        nc.vector.memset(ones_f1[:], 1.0)
        ones_1f = const.tile([1, FIN], F32, tag="ones_1f")
        nc.vector.memset(ones_1f[:], 1.0)
        ones_row = const.tile([1, 128], F32, tag="ones_row")
        nc.vector.memset(ones_row[:], 1.0)

        # z accumulator, pre-initialized with own-node features (self term)
        zu_t = wide.tile([128, S], F32, tag="zu")
        nc.sync.dma_start(out=zu_t[:], in_=xownT_d[:])
        pooled_acc = const.tile([128, GPC], F32, tag="pooled_acc")
        nc.vector.memset(pooled_acc[:], 0.0)

        # =========================== code MLP branch (fills bubbles)
        nbl = (GPC + 127) // 128
        code_nm = const.tile([128, nbl * D], F32, tag="code_nm")
        nc.sync.dma_start(
            out=code_nm[:].rearrange("p (b f) -> p b f", b=nbl),
            in_=code_d[:].rearrange("(b p) f -> p b f", p=128))
        codeT = const.tile([128, GPC], F32, tag="codeT")
        for b in range(nbl):
            tp = pp.tile([128, 128], F32, tag="tp")
            nc.tensor.transpose(out=tp[:], in_=code_nm[:, b * D:(b + 1) * D],
                                identity=ident_s[:])
            nc.vector.tensor_copy(out=codeT[:, b * 128:(b + 1) * 128], in_=tp[:])
        cps = pp3.tile([128, GPC], F32, tag="zp")
        nc.tensor.matmul(out=cps[:], lhsT=ws["fc1_w"][:], rhs=codeT[:],
                         start=True, stop=True)
        c1_s = const.tile([128, GPC], F32, tag="c1_s")
        nc.scalar.activation(out=c1_s[:], in_=cps[:], func=AF.Relu,
                             bias=ws["fc1_b"][:, :1])
        cps2 = pp3.tile([128, GPC], F32, tag="zp")
        nc.tensor.matmul(out=cps2[:], lhsT=ws["fc2_w"][:], rhs=c1_s[:],
                         start=True, stop=True)
        c2_s = const.tile([128, GPC], F32, tag="c2_s")
        nc.scalar.activation(out=c2_s[:], in_=cps2[:], func=AF.Relu,
                             bias=ws["fc2_b"][:, :1])
        cps3 = pp.tile([OUT, GPC], F32, tag="up")
        nc.tensor.matmul(out=cps3[:], lhsT=ws["fc3_w"][:], rhs=c2_s[:],
                         start=True, stop=True)
        c3_s = const.tile([OUT, GPC], F32, tag="c3_s")
        nc.scalar.activation(out=c3_s[:], in_=cps3[:], func=AF.Identity,
                             bias=ws["fc3_b"][:, :1])
        e64 = const.tile([OUT, GPC], F32, tag="e64")
        nc.scalar.activation(out=e64[:], in_=c3_s[:], func=AF.Exp)
        lsp = pp.tile([1, GPC], F32, tag="tp")
        nc.tensor.matmul(out=lsp[:], lhsT=ones_d1[:], rhs=e64[:],
                         start=True, stop=True)
        lse_s = const.tile([1, GPC], F32, tag="lse_s")
        nc.scalar.activation(out=lse_s[:], in_=lsp[:], func=AF.Ln)
        bcp = pp.tile([OUT, GPC], F32, tag="up")
        nc.tensor.matmul(out=bcp[:], lhsT=ones_1d[:], rhs=lse_s[:],
                         start=True, stop=True)
        code_embT = const.tile([OUT, GPC], F32, tag="code_embT")
        nc.vector.tensor_tensor(out=code_embT[:], in0=c3_s[:], in1=bcp[:],
                                op=ALU.subtract)

        # =========================== GIN convs
        idxcol = {}
        for li, cv_ in enumerate(plan["conv"]):
            i_s = const.tile([128, cv_["POS"] // 16], I16, tag=f"idx{li+1}")
            nc.sync.dma_start(out=i_s[:], in_=idx_d[li][:])
            c_s = const.tile([128, cv_["C"]], GDT, tag=f"col{li+1}")
            nc.sync.dma_start(out=c_s[:], in_=col_d[li][:])
            idxcol[li + 1] = (i_s, c_s)

        def conv(idx, cv, src_views, w1_s, b1_s, gam_s, bet_s,
                 pad_u, ari, aro, pass_deps=None):
            """Chunk loop with interleaved per-bin layer1, then BN stats +
            AllReduce + BN params.  Returns (sc, sh) tiles.
            src_views: per-pass DRAM APs.  pad_u: [128,1] expected layer1
            value of pad columns (subtracted npad times from the stats).
            pass_deps: per-pass instruction the first gather must wait on."""
            C, POS = cv["C"], cv["POS"]
            chunks, ops = cv["chunks"], cv["ops"]
            idx_s, col_s = idxcol[idx]
            ssum = const.tile([128, nG], F32, tag=f"ssum{idx}")
            ssq = const.tile([128, nG], F32, tag=f"ssq{idx}")

            # map chunk -> (op index, block within op)
            chunk_op = []
            for oi, (r, plo, ni) in enumerate(ops):
                for b in range(ni // 128):
                    chunk_op.append((oi, b))
            assert len(chunk_op) == C

            # per bin: the (bin, pass) group whose close should emit layer1
            last_group_of_bin = {}
            for ci_, (r, sb) in enumerate(chunks):
                last_group_of_bin[sb // 4] = (sb // 4, r)

            def layer1(g):
                cols = slice(g * 512, (g + 1) * 512)
                up = pp.tile([128, 512], F32, tag="up")
                nc.tensor.matmul(out=up[:], lhsT=w1_s[:], rhs=zu_t[:, cols],
                                 start=True, stop=True)
                nc.scalar.activation(out=zu_t[:, cols], in_=up[:],
                                     func=AF.Identity, bias=b1_s[:, :1],
                                     accum_out=ssum[:, g:g + 1])
                sq = work.tile([128, 512], F32, tag="sq")
                nc.scalar.activation(out=sq[:], in_=zu_t[:, cols],
                                     func=AF.Square,
                                     accum_out=ssq[:, g:g + 1])

            gtiles = {}
            stiles = {}
            cur_group = None       # (bin, pass)
            zp = None
            first_op_of_pass = {}
            for oi, (r, plo, ni) in enumerate(ops):
                if r not in first_op_of_pass:
                    first_op_of_pass[r] = oi

            def close_group():
                nonlocal cur_group, zp
                if cur_group is None:
                    return
                bn = cur_group[0]
                cols = slice(bn * 512, (bn + 1) * 512)
                nc.vector.tensor_tensor(out=zu_t[:, cols], in0=zu_t[:, cols],
                                        in1=zp[:], op=ALU.add)
                if last_group_of_bin.get(bn) == cur_group:
                    layer1(bn)
                cur_group, zp = None, None

            for ci in range(C):
                r, sb = chunks[ci]
                bn, sl4 = sb // 4, sb % 4
                oi, blk = chunk_op[ci]
                if oi not in gtiles:
                    opr, plo, ni = ops[oi]
                    gt = gwork.tile([128, c.NIMAX], GDT, tag="gt")
                    g_ins = nc.gpsimd.dma_gather(
                        gt[:, :ni].rearrange("p (k f) -> p k f", k=ni // 128),
                        src_views[opr],
                        idx_s[:, plo // 16:(plo + ni) // 16],
                        ni, ni, 128, elem_step=D,
                        single_packet=False, queue_num=oi % c.NQ)
                    if pass_deps is not None and oi == first_op_of_pass[opr]:
                        add_dep_helper(g_ins.ins, pass_deps[opr].ins, True,
                                       "gather after AG")
                    gtiles = {oi: gt}
                if ci % IOB == 0:
                    nob = min(IOB, C - ci)
                    st = swork.tile([128, IOB * 128], GDT, tag="st")
                    nc.vector.tensor_tensor(
                        out=st[:, :nob * 128].rearrange("p (c f) -> p c f", c=nob),
                        in0=col_s[:, ci:ci + nob].to_broadcast([128, nob, 128]),
                        in1=iota_s[:, :nob * 128].rearrange("p (c f) -> p c f", c=nob),
                        op=ALU.is_equal)
                    stiles = {ci // IOB: st}
                if cur_group != (bn, r):
                    close_group()
                    cur_group = (bn, r)
                    zp = pp3.tile([128, 512], F32, tag="zp")
                is_first = (ci == 0 or chunks[ci - 1][0] != r
                            or chunks[ci - 1][1] // 4 != bn)
                is_last = (ci == C - 1 or chunks[ci + 1][0] != chunks[ci][0]
                           or chunks[ci + 1][1] // 4 != bn)
                nc.tensor.matmul(
                    out=zp[:, sl4 * 128:(sl4 + 1) * 128],
                    lhsT=gtiles[oi][:, blk * 128:(blk + 1) * 128],
                    rhs=stiles[ci // IOB][:, (ci % IOB) * 128:(ci % IOB + 1) * 128],
                    start=is_first, stop=is_last,
                    skip_group_check=True)
            close_group()

            # ---- BN stats + AllReduce
            sum_r = const.tile([128, 1], F32, tag=f"sum_r{idx}")
            ssq_r = const.tile([128, 1], F32, tag=f"ssq_r{idx}")
            nc.vector.tensor_reduce(out=sum_r[:], in_=ssum[:],
                                    axis=mybir.AxisListType.X, op=ALU.add)
            nc.vector.tensor_reduce(out=ssq_r[:], in_=ssq[:],
                                    axis=mybir.AxisListType.X, op=ALU.add)
            usq = const.tile([128, 1], F32, tag=f"usq{idx}")
            nc.scalar.activation(out=usq[:], in_=pad_u[:], func=AF.Square)
            tmp1 = const.tile([128, 1], F32, tag=f"tmp1_{idx}")
            nc.vector.tensor_tensor(out=tmp1[:], in0=pad_u[:], in1=nh_s[:],
                                    op=ALU.mult)
            nc.vector.tensor_tensor(out=sum_r[:], in0=sum_r[:], in1=tmp1[:],
                                    op=ALU.subtract)
            nc.vector.tensor_tensor(out=tmp1[:], in0=usq[:], in1=nh_s[:],
                                    op=ALU.mult)
            nc.vector.tensor_tensor(out=ssq_r[:], in0=ssq_r[:], in1=tmp1[:],
                                    op=ALU.subtract)
            pack = const.tile([128, 2], F32, tag=f"pack{idx}")
            nc.vector.tensor_copy(out=pack[:, 0:1], in_=sum_r[:])
            nc.vector.tensor_copy(out=pack[:, 1:2], in_=ssq_r[:])
            nc.sync.dma_start(out=ari[:], in_=pack[:])
            ar = nc.gpsimd.collective_compute(
                "AllReduce", ALU.add, replica_groups=rg,
                ins=[ari[:]], outs=[aro[:]])
            rb = const.tile([128, 2], F32, tag=f"rb{idx}")
            d = nc.sync.dma_start(out=rb[:], in_=aro[:])
            add_dep_helper(d.ins, ar.ins, True, "read after AR")
            mean = const.tile([128, 1], F32, tag=f"mean{idx}")
            m2 = const.tile([128, 1], F32, tag=f"m2{idx}")
            nc.scalar.activation(out=mean[:], in_=rb[:, 0:1], func=AF.Copy,
                                 scale=1.0 / c.N)
            nc.scalar.activation(out=m2[:], in_=rb[:, 1:2], func=AF.Copy,
                                 scale=1.0 / c.N)
            msq = const.tile([128, 1], F32, tag=f"msq{idx}")
            nc.scalar.activation(out=msq[:], in_=mean[:], func=AF.Square)
            var = const.tile([128, 1], F32, tag=f"var{idx}")
            nc.vector.tensor_tensor(out=var[:], in0=m2[:], in1=msq[:],
                                    op=ALU.subtract)
            nc.vector.tensor_scalar_add(out=var[:], in0=var[:], scalar1=BN_EPS)
            std = const.tile([128, 1], F32, tag=f"std{idx}")
            nc.scalar.activation(out=std[:], in_=var[:], func=AF.Sqrt)
            inv = const.tile([128, 1], F32, tag=f"inv{idx}")
            nc.vector.reciprocal(out=inv[:], in_=std[:])
            sc = const.tile([128, 1], F32, tag=f"sc{idx}")
            nc.vector.tensor_tensor(out=sc[:], in0=gam_s[:], in1=inv[:],
                                    op=ALU.mult)
            sh = const.tile([128, 1], F32, tag=f"sh{idx}")
            nc.vector.tensor_tensor(out=sh[:], in0=mean[:], in1=sc[:],
                                    op=ALU.mult)
            nc.vector.tensor_tensor(out=sh[:], in0=bet_s[:], in1=sh[:],
                                    op=ALU.subtract)
            return sc, sh

        cvs = plan["conv"]

        # ---- conv1
        src_views1 = [xg_d[int(r1_lo[q]):int(r1_lo[q + 1]), :]
                      for q in range(c.NR)]
        sc1, sh1 = conv(1, cvs[0], src_views1,
                        ws["c1_w1"], ws["c1_b1"], ws["c1_gamma"], ws["c1_beta"],
                        ws["c1_b1"], ar_in[0], ar_out[0])

        # ---- conv1 tail per slot-quarter: BN apply, layer2 (feat-major,
        # h1 written in place into zu_t), transposes, h1loc DMA, AllGather
        ag_list = []
        bin_lo = 0
        for q in range(c.NR):
            dmas = []
            for g in range(bin_lo, bin_lo + nbq[q]):
                cols = slice(g * 512, (g + 1) * 512)
                nc.scalar.activation(out=zu_t[:, cols], in_=zu_t[:, cols],
                                     func=AF.Relu, bias=sh1[:, :1],
                                     scale=sc1[:, :1])
                hp = pp.tile([128, 512], F32, tag="up")
                nc.tensor.matmul(out=hp[:], lhsT=ws["c1_w2"][:],
                                 rhs=zu_t[:, cols], start=True, stop=True)
                nc.scalar.activation(out=zu_t[:, cols], in_=hp[:],
                                     func=AF.Relu, bias=ws["c1_b2"][:, :1])
                hnm = work.tile([128, 4 * D], GDT, tag="hnm")
                for t in range(4):
                    tp = pp.tile([128, 128], F32, tag="tp")
                    nc.tensor.transpose(
                        out=tp[:], in_=zu_t[:, g * 512 + t * 128:
                                            g * 512 + (t + 1) * 128],
                        identity=ident_s[:])
                    nc.vector.tensor_copy(out=hnm[:, t * D:(t + 1) * D],
                                          in_=tp[:])
                d = nc.sync.dma_start(
                    out=h1loc_d[g * 512:(g + 1) * 512, :].rearrange(
                        "(b p) f -> p b f", p=128),
                    in_=hnm[:].rearrange("p (b f) -> p b f", b=4))
                dmas.append(d)
            ag = nc.gpsimd.collective_compute(
                "AllGather", ALU.bypass, replica_groups=rg,
                ins=[h1loc_d[int(qslot_lo[q]):int(qslot_lo[q + 1]), :]],
                outs=[h1all_d[q][:]])
            for d in dmas:
                add_dep_helper(ag.ins, d.ins, True, "AG after h1loc")
            ag_list.append(ag)
            bin_lo += nbq[q]

        # pad-column layer1 value for conv2 stats correction:
        # q = c2_w1^T relu(c1_w2^T relu(sc1*c1_b1 + sh1) + c1_b2) + c2_b1
        cvec = const.tile([128, 1], F32, tag="cvec")
        nc.scalar.activation(out=cvec[:], in_=ws["c1_b1"][:], func=AF.Relu,
                             bias=sh1[:, :1], scale=sc1[:, :1])
        pv = pp.tile([128, 1], F32, tag="tp")
        nc.tensor.matmul(out=pv[:], lhsT=ws["c1_w2"][:], rhs=cvec[:],
                         start=True, stop=True)
        pvec = const.tile([128, 1], F32, tag="pvec")
        nc.scalar.activation(out=pvec[:], in_=pv[:], func=AF.Relu,
                             bias=ws["c1_b2"][:, :1])
        qv = pp.tile([128, 1], F32, tag="tp")
        nc.tensor.matmul(out=qv[:], lhsT=ws["c2_w1"][:], rhs=pvec[:],
                         start=True, stop=True)
        qvec = const.tile([128, 1], F32, tag="qvec")
        nc.scalar.activation(out=qvec[:], in_=qv[:], func=AF.Identity,
                             bias=ws["c2_b1"][:, :1])

        # ---- conv2 (gathers wait per-pass on the matching AllGather)
        src_views2 = [h1all_d[q][:] for q in range(c.NR)]
        sc2, sh2 = conv(2, cvs[1], src_views2,
                        ws["c2_w1"], ws["c2_b1"], ws["c2_gamma"], ws["c2_beta"],
                        qvec, ar_in[1], ar_out[1], pass_deps=ag_list)

        # ---- conv2 tail: BN apply + node-major layer2 + pooling per bin
        for g in range(nbin):
            cols = slice(g * 512, (g + 1) * 512)
            nc.scalar.activation(out=zu_t[:, cols], in_=zu_t[:, cols],
                                 func=AF.Relu, bias=sh2[:, :1],
                                 scale=sc2[:, :1])
            for t in range(4):
                k = g * 4 + t
                hpT = pp.tile([128, 128], F32, tag="tp")
                nc.tensor.matmul(
                    out=hpT[:],
                    lhsT=zu_t[:, g * 512 + t * 128:g * 512 + (t + 1) * 128],
                    rhs=ws["c2_w2"][:], start=True, stop=False,
                    skip_group_check=True)
                nc.tensor.matmul(out=hpT[:], lhsT=ones_row[:], rhs=b2r_s[:],
                                 start=False, stop=True, skip_group_check=True)
                hT = work.tile([128, 128], F32, tag="hT")
                nc.scalar.activation(out=hT[:], in_=hpT[:], func=AF.Relu)
                lo = int(win_lo[k])
                poolw = pp.tile([128, c.GW], F32, tag="up")
                nc.tensor.matmul(out=poolw[:], lhsT=hT[:],
                                 rhs=pmat_s[:, k * c.GW:(k + 1) * c.GW],
                                 start=True, stop=True)
                nc.vector.tensor_tensor(
                    out=pooled_acc[:, lo:lo + c.GW],
                    in0=pooled_acc[:, lo:lo + c.GW],
                    in1=poolw[:], op=ALU.add)

        # =========================== head
        hd1 = pp3.tile([128, GPC], F32, tag="zp")
        nc.tensor.matmul(out=hd1[:], lhsT=ws["g_l1_w"][:], rhs=pooled_acc[:],
                         start=True, stop=True)
        t_s = const.tile([128, GPC], F32, tag="t_s")
        nc.scalar.activation(out=t_s[:], in_=hd1[:], func=AF.Relu,
                             bias=ws["g_l1_b"][:, :1])
        hd2 = pp.tile([OUT, GPC], F32, tag="up")
        nc.tensor.matmul(out=hd2[:], lhsT=ws["g_l2_w"][:], rhs=t_s[:],
                         start=True, stop=True)
        trans_embT = const.tile([OUT, GPC], F32, tag="trans_embT")
        nc.scalar.activation(out=trans_embT[:], in_=hd2[:], func=AF.Identity,
                             bias=ws["g_l2_b"][:, :1])
        fp = pp.tile([FIN, GPC], F32, tag="tp")
        nc.tensor.matmul(out=fp[:], lhsT=ws["fin_w"][0:OUT, :],
                         rhs=code_embT[:], start=True, stop=False,
                         skip_group_check=True)
        nc.tensor.matmul(out=fp[:], lhsT=finw_hi[:],
                         rhs=trans_embT[:], start=False, stop=True,
                         skip_group_check=True)
        f_s = const.tile([FIN, GPC], F32, tag="f_s")
        nc.scalar.activation(out=f_s[:], in_=fp[:], func=AF.Identity,
                             bias=ws["fin_b"][:, :1])
        ef = const.tile([FIN, GPC], F32, tag="ef")
        nc.scalar.activation(out=ef[:], in_=f_s[:], func=AF.Exp)
        lfp = pp.tile([1, GPC], F32, tag="up")
        nc.tensor.matmul(out=lfp[:], lhsT=ones_f1[:], rhs=ef[:],
                         start=True, stop=True)
        lf_s = const.tile([1, GPC], F32, tag="lf_s")
        nc.scalar.activation(out=lf_s[:], in_=lfp[:], func=AF.Ln)
        bfp = pp3.tile([FIN, GPC], F32, tag="zp")
        nc.tensor.matmul(out=bfp[:], lhsT=ones_1f[:], rhs=lf_s[:],
                         start=True, stop=True)
        outT = const.tile([FIN, GPC], F32, tag="outT")
        nc.vector.tensor_tensor(out=outT[:], in0=f_s[:], in1=bfp[:],
                                op=ALU.subtract)
        nc.sync.dma_start(out=out_d[:], in_=outT[:])

    if not nc.is_finalized():
        nc.finalize()
    return nc


# ---------------------------------------------------------------- runner

def make_in_maps(inputs, plan, cfg):
    c = cfg
    wnames = ["c1_w1", "c1_b1", "c1_gamma", "c1_beta", "c1_w2", "c1_b2",
              "c2_w1", "c2_b1", "c2_gamma", "c2_beta", "c2_w2", "c2_b2",
              "g_l1_w", "g_l1_b", "g_l2_w", "g_l2_b",
              "fc1_w", "fc1_b", "fc2_w", "fc2_b", "fc3_w", "fc3_b",
              "fin_w", "fin_b"]
    np_gdt = np.float32 if c.GDT == F32 else __import__("ml_dtypes").bfloat16
    x = np.asarray(inputs["x"], np.float32)
    xg = x.astype(np_gdt)
    S = plan["S"]
    node_lo = plan["node_lo"]
    code = np.ascontiguousarray(np.asarray(inputs["code_x"], np.float32))
    ident = np.eye(128, dtype=np.float32)
    b2r = np.ascontiguousarray(
        np.asarray(inputs["c2_b2"], np.float32).reshape(1, c.D))
    in_maps = []
    for ci in range(c.W):
        lo, hi = int(node_lo[ci]), int(node_lo[ci + 1])
        xownT = np.zeros((128, S), np.float32)
        xownT[:, :hi - lo] = x[lo:hi].T
        m = {
            "xg": xg,
            "xownT": xownT,
            "pmat": plan["pmats"][ci],
            "code": code[ci * c.GPC:(ci + 1) * c.GPC],
            "ident": ident,
            "nh": np.full((128, 1), float(S - plan["n_real"][ci]), np.float32),
            "c2_b2r": b2r,
        }
        for li in (0, 1):
            cv = plan["conv"][li]
            m[f"idx{li}"] = cv["cores"][ci]["idx16"]
            m[f"col{li}"] = cv["cores"][ci]["colidx"].astype(np_gdt)
        for k in wnames:
            m[k] = np.ascontiguousarray(np.asarray(inputs[k], np.float32))
        in_maps.append(m)
    return in_maps


_CACHE = {}


def _get_compiled(inputs, cfg):
    if "prog" not in _CACHE:
        plan = _plan(inputs["edge_index"], inputs["batch"], cfg)
        nc = _build(plan, cfg)
        _CACHE["prog"] = (plan, nc)
    return _CACHE["prog"]


def kernel(**inputs) -> np.ndarray:
    cfg = DEFAULT_CFG
    plan, nc = _get_compiled(inputs, cfg)
    in_maps = make_in_maps(inputs, plan, cfg)
    res = run_bass_kernel_spmd(nc, in_maps, core_ids=list(range(cfg.W)))
    outs = [res.results[ci]["out"].T for ci in range(cfg.W)]
    return np.ascontiguousarray(np.concatenate(outs, axis=0).astype(np.float32))


# revision 18
# speedup vs baseline: 1.1922x; 1.0452x over previous
"""Trainium2 Bass kernel for the GIN message-passing model (8 NeuronCores).

Sharding: graph partitioning.  Core c owns graphs [c*G/8, (c+1)*G/8) and the
contiguous node range of those graphs (batch is sorted), plus every edge whose
dst lands there.  dst nodes get compact slot ranks.

Aggregation: edges gathered with `dma_gather` (int16 indices => source row
space split into 4 contiguous ranges = 4 passes).  Gathers cycle across 4
SWDGE queues (ops of <=2048 idxs) so SDMA descriptor drains overlap ~4x.
Within a pass edges are dst-sorted and packed into 128-position chunks
aligned to 128-slot subbins; a chunk's segment-sum is one matmul (gathered
rows stationary, on-chip one-hot moving) into the subbin's slice of a
512-slot PSUM bin.  Self-edges are NOT gathered: z is pre-initialized with
the node's own features (host-transposed x for conv1; in-place h1 for conv2).

conv layer-1 matmuls are interleaved per-bin into the chunk stream so they
overlap the gather tail.  conv2's 4 source ranges are slot-quarters of each
core's h1, published by 4 pipelined AllGathers so conv2 pass-q gathers start
as soon as AG_q lands.  conv2's layer-2 runs node-major (no transposes),
feeding pooling matmuls directly; BN pad corrections are computed on device.
"""

import os
import sys

for _p in ("/opt/trn_rl_repo",):
    if _p not in sys.path:
        sys.path.insert(0, _p)

KDBG = int(os.environ.get("KDBG", "0"))

import numpy as np
from contextlib import ExitStack

import concourse.bass as bass
import concourse.bacc as bacc
import concourse.mybir as mybir
import concourse.tile as tile
from concourse.bass_utils import run_bass_kernel_spmd
from concourse.tile_rust import add_dep_helper

F32 = mybir.dt.float32
BF16 = mybir.dt.bfloat16
I32 = mybir.dt.int32
I16 = mybir.dt.int16
AF = mybir.ActivationFunctionType
ALU = mybir.AluOpType

BN_EPS = 1e-5
PADCOL = 200.0          # colidx value for pad positions (never matches 0..127)


class Cfg:
    def __init__(self, N=100000, E=500000, G=2048, D=128, OUT=64, FIN=2, W=8,
                 NR=4, NIMAX=2048, NQ=4, GW=32, GDT=BF16):
        self.N, self.E, self.G, self.D, self.OUT, self.FIN, self.W = N, E, G, D, OUT, FIN, W
        self.NR = NR        # source ranges (int16 index limit)
        self.NIMAX = NIMAX  # max positions per dma_gather
        self.NQ = NQ        # SWDGE queues to cycle gathers over
        self.GW = GW        # pooling window width (graphs)
        self.GDT = GDT      # gather dtype (bf16 or f32)
        self.GPC = G // W   # graphs per core


DEFAULT_CFG = Cfg()


def _wrap_idx(lst):
    """dma_gather index layout: position j is read from row j%16, col j//16."""
    assert len(lst) % 16 == 0
    return np.tile(np.asarray(lst, np.int16).reshape(-1, 16).T, (8, 1))


# ---------------------------------------------------------------- host plan

def _plan(edge_index, batch, cfg):
    c = cfg
    batch = np.asarray(batch).astype(np.int64)
    ei = np.asarray(edge_index).astype(np.int64)
    owner = (batch // c.GPC).astype(np.int64)

    src, dst = ei[0], ei[1]          # no synthetic self-edges
    eowner = owner[dst]

    # compact slot ranks per core
    n_real = np.zeros(c.W, np.int64)
    slot_of = np.full(c.N, -1, np.int64)
    node_lo = np.zeros(c.W + 1, np.int64)
    for ci in range(c.W):
        node_lo[ci] = np.searchsorted(batch, ci * c.GPC)
    node_lo[c.W] = c.N
    for ci in range(c.W):
        lo, hi = node_lo[ci], node_lo[ci + 1]
        n_real[ci] = hi - lo
        slot_of[lo:hi] = np.arange(hi - lo)
    S = int(((n_real.max() + 511) // 512) * 512)
    nbin = S // 512
    nsub = S // 128
    nSC = S // 128

    # conv1 source ranges: contiguous quarters of the x row space
    r1_lo = np.array([(c.N * q) // c.NR for q in range(c.NR + 1)], np.int64)
    assert (np.diff(r1_lo) <= 32767).all()
    epass1 = np.searchsorted(r1_lo[1:], src, side="right")
    loc1 = src - r1_lo[epass1]

    # conv2 source ranges: (slot-half, core-group-of-4).  Each half of the
    # slot space is published by one AllGather; within a half the 4 ranges
    # split by core group so every range stays under the int16 limit.
    nbh = [(nbin + 1) // 2, nbin // 2]
    hslot_lo = np.array([0, nbh[0] * 512, S], np.int64)
    Hsz = np.diff(hslot_lo)
    assert (4 * Hsz <= 32767).all()
    s_src = slot_of[src]
    o_src = owner[src]
    half = (s_src >= hslot_lo[1]).astype(np.int64)
    epass2 = 2 * half + o_src // 4
    loc2 = (o_src % 4) * Hsz[half] + (s_src - hslot_lo[half])

    def build_conv(epass, loc):
        """epass: per-edge source range id; loc: row within that range.
        Returns common chunk structure + per-core idx/colidx arrays."""
        counts = np.zeros((c.W, c.NR, nsub), np.int64)
        percore_edges = []
        for ci in range(c.W):
            m = eowner == ci
            sl = slot_of[dst[m]]
            pr = epass[m]
            rows = loc[m]
            sub = sl // 128
            order = np.lexsort((sl, sub, pr))
            sl, pr, rows, sub = sl[order], pr[order], rows[order], sub[order]
            np.add.at(counts[ci], (pr, sub), 1)
            percore_edges.append((sl, pr, rows, sub))
        # common chunk structure (max over cores per segment).  Even empty
        # segments get one (all-pad) chunk: its matmul zero-initializes the
        # PSUM slice that the per-bin fold reads (else stale PSUM garbage
        # lands in pad slots and poisons the BN stats).
        nch = np.maximum(1, -(-counts.max(axis=0) // 128))  # [NR, nsub]
        chunks = []     # (pass, subbin)
        for r in range(c.NR):
            for sb in range(nsub):
                for k in range(int(nch[r, sb])):
                    chunks.append((r, sb))
        C = len(chunks)
        POS = C * 128
        pass_pos_lo = np.zeros(c.NR + 1, np.int64)
        for r in range(c.NR):
            pass_pos_lo[r + 1] = pass_pos_lo[r] + 128 * int(nch[r].sum())
        # per-core arrays
        cores = []
        for ci in range(c.W):
            sl, pr, rows, sub = percore_edges[ci]
            idx_local = np.zeros(POS, np.int64)          # pad -> row 0 of range
            colv = np.full((128, C), PADCOL, np.float64)
            seg_base = {}
            pos = 0
            for r in range(c.NR):
                for sb in range(nsub):
                    seg_base[(r, sb)] = pos
                    pos += 128 * int(nch[r, sb])
            key = pr * nsub + sub
            uniq, start_idx = np.unique(key, return_index=True)
            end_idx = np.append(start_idx[1:], len(key))
            for u, s0, s1 in zip(uniq, start_idx, end_idx):
                r, sb = int(u) // nsub, int(u) % nsub
                base_ = seg_base[(r, sb)]
                n = s1 - s0
                p = base_ + np.arange(n)
                idx_local[p] = rows[s0:s1]
                colv[p % 128, p // 128] = sl[s0:s1] - sb * 128
            wrapped = [
                _wrap_idx(idx_local[pass_pos_lo[r]:pass_pos_lo[r + 1]])
                for r in range(c.NR) if pass_pos_lo[r + 1] > pass_pos_lo[r]
            ]
            idx16 = np.concatenate(wrapped, axis=1) if wrapped else np.zeros((128, 0), np.int16)
            cores.append(dict(idx16=idx16, colidx=colv))
        # gather op list: per pass, ops of <= NIMAX positions
        ops = []        # (pass, pos_lo, ni)
        for r in range(c.NR):
            p0, p1 = int(pass_pos_lo[r]), int(pass_pos_lo[r + 1])
            while p0 < p1:
                ni = min(c.NIMAX, p1 - p0)
                ops.append((r, p0, ni))
                p0 += ni
        return dict(C=C, POS=POS, chunks=chunks, ops=ops, cores=cores)

    conv1 = build_conv(epass1, loc1)
    conv2 = build_conv(epass2, loc2)

    # pooling plan
    gos_all = []
    for ci in range(c.W):
        gos = np.full(S, -1, np.int64)
        lo, hi = node_lo[ci], node_lo[ci + 1]
        gos[:hi - lo] = batch[lo:hi] - ci * c.GPC
        gos_all.append(gos)
    win_lo = np.zeros(nSC, np.int64)
    prev = 0
    for k in range(nSC):
        lo_k, hi_k = c.GPC, -1
        for gos in gos_all:
            seg = gos[k * 128:(k + 1) * 128]
            v = seg[seg >= 0]
            if len(v):
                lo_k = min(lo_k, int(v.min()))
                hi_k = max(hi_k, int(v.max()))
        if hi_k < 0:
            lo_k = hi_k = min(prev, c.GPC - 1)
        assert hi_k - lo_k + 1 <= c.GW, f"pool window too wide: {lo_k}..{hi_k}"
        lo_k = max(0, min(lo_k, c.GPC - c.GW))
        assert lo_k <= prev + c.GW, "pool window coverage gap"
        win_lo[k] = lo_k
        prev = max(prev, lo_k + c.GW - 1)
    covered = np.zeros(c.GPC, bool)
    for k in range(nSC):
        covered[win_lo[k]:win_lo[k] + c.GW] = True
    assert covered.all()

    pmats = []
    for ci in range(c.W):
        pmat = np.zeros((128, nSC * c.GW), np.float32)
        gos = gos_all[ci]
        for k in range(nSC):
            seg = gos[k * 128:(k + 1) * 128]
            for p in range(128):
                if seg[p] >= 0:
                    w = int(seg[p] - win_lo[k])
                    pmat[p, k * c.GW + w] = 1.0
        pmats.append(pmat)

    return dict(S=S, nbin=nbin, nSC=nSC, win_lo=win_lo, conv=[conv1, conv2],
                n_real=n_real, node_lo=node_lo, r1_lo=r1_lo,
                hslot_lo=hslot_lo, Hsz=Hsz, nbh=nbh, pmats=pmats)


# ---------------------------------------------------------------- program

def _build(plan, cfg):
    c = cfg
    S, nbin, nSC = plan["S"], plan["nbin"], plan["nSC"]
    win_lo = plan["win_lo"]
    hslot_lo, Hsz, nbh = plan["hslot_lo"], plan["Hsz"], plan["nbh"]
    r1_lo = plan["r1_lo"]
    D, OUT, FIN, GPC = c.D, c.OUT, c.FIN, c.GPC
    rg = [list(range(c.W))]
    nG = nbin
    GDT = c.GDT

    nc = bacc.Bacc(num_devices=c.W, num_swdge_queues=c.NQ)

    # ---- external inputs
    xg_d = nc.dram_tensor("xg", [c.N, D], GDT, kind="ExternalInput")
    xownT_d = nc.dram_tensor("xownT", [128, S], F32, kind="ExternalInput")
    pmat_d = nc.dram_tensor("pmat", [128, nSC * c.GW], F32, kind="ExternalInput")
    idx_d, col_d = [], []
    for li in (0, 1):
        cv = plan["conv"][li]
        idx_d.append(nc.dram_tensor(f"idx{li}", [128, cv["POS"] // 16], I16,
                                    kind="ExternalInput"))
        col_d.append(nc.dram_tensor(f"col{li}", [128, cv["C"]], GDT,
                                    kind="ExternalInput"))
    code_d = nc.dram_tensor("code", [GPC, D], F32, kind="ExternalInput")
    ident_d = nc.dram_tensor("ident", [128, 128], F32, kind="ExternalInput")
    nh_d = nc.dram_tensor("nh", [128, 1], F32, kind="ExternalInput")

    wspec = {
        "c1_w1": [D, D], "c1_b1": [D], "c1_gamma": [D], "c1_beta": [D],
        "c1_w2": [D, D], "c1_b2": [D],
        "c2_w1": [D, D], "c2_b1": [D], "c2_gamma": [D], "c2_beta": [D],
        "c2_w2": [D, D], "c2_b2": [D],
        "g_l1_w": [D, D], "g_l1_b": [D], "g_l2_w": [D, OUT], "g_l2_b": [OUT],
        "fc1_w": [D, D], "fc1_b": [D], "fc2_w": [D, D], "fc2_b": [D],
        "fc3_w": [D, OUT], "fc3_b": [OUT],
        "fin_w": [2 * OUT, FIN], "fin_b": [FIN],
    }
    wd = {k: nc.dram_tensor(k, v, F32, kind="ExternalInput") for k, v in wspec.items()}
    b2r_d = nc.dram_tensor("c2_b2r", [1, D], F32, kind="ExternalInput")

    out_d = nc.dram_tensor("out", [FIN, GPC], F32, kind="ExternalOutput")
    dbg = {}
    if KDBG:
        for nm, shp in [("d_pack1", [128, 2]), ("d_pack2", [128, 2]),
                        ("d_bn1", [128, 2]), ("d_bn2", [128, 2]),
                        ("d_u1", [128, 512]), ("d_u2", [128, 512]),
                        ("d_h1", [128, 512]), ("d_pooled", [128, GPC])]:
            dbg[nm] = nc.dram_tensor(nm, shp, F32, kind="ExternalOutput")

    # ---- internal DRAM
    h1loc_d = nc.dram_tensor("h1loc", [S, D], GDT)
    h1all_d = [nc.dram_tensor(f"h1all{h}", [c.W * int(Hsz[h]), D], GDT,
                              addr_space="Shared") for h in range(2)]
    ar_in = [nc.dram_tensor(f"ar{i}i", [128, 2], F32) for i in (1, 2)]
    ar_out = [nc.dram_tensor(f"ar{i}o", [128, 2], F32, addr_space="Shared")
              for i in (1, 2)]

    with tile.TileContext(nc) as tc, ExitStack() as ctx:
        const = ctx.enter_context(tc.tile_pool(name="const", bufs=1))
        work = ctx.enter_context(tc.tile_pool(name="work", bufs=3))
        gwork = ctx.enter_context(tc.tile_pool(name="gwork", bufs=12))
        swork = ctx.enter_context(tc.tile_pool(name="swork", bufs=6))
        wide = ctx.enter_context(tc.tile_pool(name="wide", bufs=1))
        pp = ctx.enter_context(tc.tile_pool(name="pp", bufs=2, space="PSUM"))
        pp3 = ctx.enter_context(tc.tile_pool(name="pp3", bufs=3, space="PSUM"))

        def cload(dram_ap, shape, dtype, tag):
            t = const.tile(shape, dtype, tag=tag)
            nc.sync.dma_start(out=t[:], in_=dram_ap)
            return t

        ident_s = cload(ident_d[:], [128, 128], F32, "ident")
        nh_s = cload(nh_d[:], [128, 1], F32, "nh")
        pmat_s = cload(pmat_d[:], [128, nSC * c.GW], F32, "pmat")
        b2r_s = cload(b2r_d[:], [1, D], F32, "b2r")

        ws = {}
        for k, shp in wspec.items():
            if len(shp) == 2:
                ws[k] = cload(wd[k][:], shp, F32, k)
            else:
                ws[k] = cload(wd[k][:, None], [shp[0], 1], F32, k)
        finw_hi = const.tile([OUT, FIN], F32, tag="finw_hi")
        nc.sync.dma_start(out=finw_hi[:], in_=wd["fin_w"][OUT:2 * OUT, :])

        # iota row pattern repeated (for one-hot gen), in gather dtype
        IOB = 8  # chunks per one-hot op
        iota_i = const.tile([128, IOB * 128], I32, tag="iota_i")
        nc.gpsimd.iota(iota_i[:], pattern=[[0, IOB], [1, 128]], base=0,
                       channel_multiplier=0)
        iota_s = const.tile([128, IOB * 128], GDT, tag="iota_s")
        nc.vector.tensor_copy(out=iota_s[:], in_=iota_i[:])

        ones_d1 = const.tile([OUT, 1], F32, tag="ones_d1")
        nc.vector.memset(ones_d1[:], 1.0)
        ones_1d = const.tile([1, OUT], F32, tag="ones_1d")
        nc.vector.memset(ones_1d[:], 1.0)
        ones_f1 = const.tile([FIN, 1], F32, tag="ones_f1")
        nc.vector.memset(ones_f1[:], 1.0)
        ones_1f = const.tile([1, FIN], F32, tag="ones_1f")
        nc.vector.memset(ones_1f[:], 1.0)
        ones_row = const.tile([1, 128], F32, tag="ones_row")
        nc.vector.memset(ones_row[:], 1.0)

        # z accumulator, pre-initialized with own-node features (self term)
        zu_t = wide.tile([128, S], F32, tag="zu")
        nc.sync.dma_start(out=zu_t[:], in_=xownT_d[:])
        pooled_acc = const.tile([128, GPC], F32, tag="pooled_acc")
        nc.vector.memset(pooled_acc[:], 0.0)

        # =========================== code MLP branch (fills bubbles)
        nbl = (GPC + 127) // 128
        code_nm = const.tile([128, nbl * D], F32, tag="code_nm")
        nc.sync.dma_start(
            out=code_nm[:].rearrange("p (b f) -> p b f", b=nbl),
            in_=code_d[:].rearrange("(b p) f -> p b f", p=128))
        codeT = const.tile([128, GPC], F32, tag="codeT")
        for b in range(nbl):
            tp = pp.tile([128, 128], F32, tag="tp")
            nc.tensor.transpose(out=tp[:], in_=code_nm[:, b * D:(b + 1) * D],
                                identity=ident_s[:])
            nc.vector.tensor_copy(out=codeT[:, b * 128:(b + 1) * 128], in_=tp[:])
        cps = pp3.tile([128, GPC], F32, tag="zp")
        nc.tensor.matmul(out=cps[:], lhsT=ws["fc1_w"][:], rhs=codeT[:],
                         start=True, stop=True)
        c1_s = const.tile([128, GPC], F32, tag="c1_s")
        nc.scalar.activation(out=c1_s[:], in_=cps[:], func=AF.Relu,
                             bias=ws["fc1_b"][:, :1])
        cps2 = pp3.tile([128, GPC], F32, tag="zp")
        nc.tensor.matmul(out=cps2[:], lhsT=ws["fc2_w"][:], rhs=c1_s[:],
                         start=True, stop=True)
        c2_s = const.tile([128, GPC], F32, tag="c2_s")
        nc.scalar.activation(out=c2_s[:], in_=cps2[:], func=AF.Relu,
                             bias=ws["fc2_b"][:, :1])
        cps3 = pp.tile([OUT, GPC], F32, tag="up")
        nc.tensor.matmul(out=cps3[:], lhsT=ws["fc3_w"][:], rhs=c2_s[:],
                         start=True, stop=True)
        c3_s = const.tile([OUT, GPC], F32, tag="c3_s")
        nc.scalar.activation(out=c3_s[:], in_=cps3[:], func=AF.Identity,
                             bias=ws["fc3_b"][:, :1])
        e64 = const.tile([OUT, GPC], F32, tag="e64")
        nc.scalar.activation(out=e64[:], in_=c3_s[:], func=AF.Exp)
        lsp = pp.tile([1, GPC], F32, tag="tp")
        nc.tensor.matmul(out=lsp[:], lhsT=ones_d1[:], rhs=e64[:],
                         start=True, stop=True)
        lse_s = const.tile([1, GPC], F32, tag="lse_s")
        nc.scalar.activation(out=lse_s[:], in_=lsp[:], func=AF.Ln)
        bcp = pp.tile([OUT, GPC], F32, tag="up")
        nc.tensor.matmul(out=bcp[:], lhsT=ones_1d[:], rhs=lse_s[:],
                         start=True, stop=True)
        code_embT = const.tile([OUT, GPC], F32, tag="code_embT")
        nc.vector.tensor_tensor(out=code_embT[:], in0=c3_s[:], in1=bcp[:],
                                op=ALU.subtract)

        # =========================== GIN convs
        idxcol = {}
        for li, cv_ in enumerate(plan["conv"]):
            i_s = const.tile([128, cv_["POS"] // 16], I16, tag=f"idx{li+1}")
            nc.sync.dma_start(out=i_s[:], in_=idx_d[li][:])
            c_s = const.tile([128, cv_["C"]], GDT, tag=f"col{li+1}")
            nc.sync.dma_start(out=c_s[:], in_=col_d[li][:])
            idxcol[li + 1] = (i_s, c_s)

        def conv(idx, cv, src_views, w1_s, b1_s, gam_s, bet_s,
                 pad_u, ari, aro, pass_deps=None):
            """Chunk loop with interleaved per-bin layer1, then BN stats +
            AllReduce + BN params.  Returns (sc, sh) tiles.
            src_views: per-pass DRAM APs.  pad_u: [128,1] expected layer1
            value of pad columns (subtracted npad times from the stats).
            pass_deps: per-pass instruction the first gather must wait on."""
            C, POS = cv["C"], cv["POS"]
            chunks, ops = cv["chunks"], cv["ops"]
            idx_s, col_s = idxcol[idx]
            ssum = const.tile([128, nG], F32, tag=f"ssum{idx}")
            ssq = const.tile([128, nG], F32, tag=f"ssq{idx}")

            # map chunk -> (op index, block within op)
            chunk_op = []
            for oi, (r, plo, ni) in enumerate(ops):
                for b in range(ni // 128):
                    chunk_op.append((oi, b))
            assert len(chunk_op) == C

            # per bin: the (bin, pass) group whose close should emit layer1
            last_group_of_bin = {}
            for ci_, (r, sb) in enumerate(chunks):
                last_group_of_bin[sb // 4] = (sb // 4, r)

            def layer1(g):
                cols = slice(g * 512, (g + 1) * 512)
                up = pp.tile([128, 512], F32, tag="up")
                nc.tensor.matmul(out=up[:], lhsT=w1_s[:], rhs=zu_t[:, cols],
                                 start=True, stop=True)
                nc.scalar.activation(out=zu_t[:, cols], in_=up[:],
                                     func=AF.Identity, bias=b1_s[:, :1],
                                     accum_out=ssum[:, g:g + 1])
                sq = work.tile([128, 512], F32, tag="sq")
                nc.scalar.activation(out=sq[:], in_=zu_t[:, cols],
                                     func=AF.Square,
                                     accum_out=ssq[:, g:g + 1])

            gtiles = {}
            stiles = {}
            cur_group = None       # (bin, pass)
            zp = None
            first_op_of_pass = {}
            for oi, (r, plo, ni) in enumerate(ops):
                if r not in first_op_of_pass:
                    first_op_of_pass[r] = oi

            def close_group():
                nonlocal cur_group, zp
                if cur_group is None:
                    return
                bn = cur_group[0]
                cols = slice(bn * 512, (bn + 1) * 512)
                nc.vector.tensor_tensor(out=zu_t[:, cols], in0=zu_t[:, cols],
                                        in1=zp[:], op=ALU.add)
                if last_group_of_bin.get(bn) == cur_group:
                    layer1(bn)
                cur_group, zp = None, None

            for ci in range(C):
                r, sb = chunks[ci]
                bn, sl4 = sb // 4, sb % 4
                oi, blk = chunk_op[ci]
                if oi not in gtiles:
                    opr, plo, ni = ops[oi]
                    gt = gwork.tile([128, c.NIMAX], GDT, tag="gt")
                    g_ins = nc.gpsimd.dma_gather(
                        gt[:, :ni].rearrange("p (k f) -> p k f", k=ni // 128),
                        src_views[opr],
                        idx_s[:, plo // 16:(plo + ni) // 16],
                        ni, ni, 128, elem_step=D,
                        single_packet=False, queue_num=oi % c.NQ)
                    if pass_deps is not None and oi == first_op_of_pass[opr]:
                        add_dep_helper(g_ins.ins, pass_deps[opr].ins, True,
                                       "gather after AG")
                    gtiles = {oi: gt}
                if ci % IOB == 0:
                    nob = min(IOB, C - ci)
                    st = swork.tile([128, IOB * 128], GDT, tag="st")
                    nc.vector.tensor_tensor(
                        out=st[:, :nob * 128].rearrange("p (c f) -> p c f", c=nob),
                        in0=col_s[:, ci:ci + nob].to_broadcast([128, nob, 128]),
                        in1=iota_s[:, :nob * 128].rearrange("p (c f) -> p c f", c=nob),
                        op=ALU.is_equal)
                    stiles = {ci // IOB: st}
                if cur_group != (bn, r):
                    close_group()
                    cur_group = (bn, r)
                    zp = pp3.tile([128, 512], F32, tag="zp")
                is_first = (ci == 0 or chunks[ci - 1][0] != r
                            or chunks[ci - 1][1] // 4 != bn)
                is_last = (ci == C - 1 or chunks[ci + 1][0] != chunks[ci][0]
                           or chunks[ci + 1][1] // 4 != bn)
                nc.tensor.matmul(
                    out=zp[:, sl4 * 128:(sl4 + 1) * 128],
                    lhsT=gtiles[oi][:, blk * 128:(blk + 1) * 128],
                    rhs=stiles[ci // IOB][:, (ci % IOB) * 128:(ci % IOB + 1) * 128],
                    start=is_first, stop=is_last,
                    skip_group_check=True)
            close_group()

            # ---- BN stats + AllReduce
            sum_r = const.tile([128, 1], F32, tag=f"sum_r{idx}")
            ssq_r = const.tile([128, 1], F32, tag=f"ssq_r{idx}")
            nc.vector.tensor_reduce(out=sum_r[:], in_=ssum[:],
                                    axis=mybir.AxisListType.X, op=ALU.add)
            nc.vector.tensor_reduce(out=ssq_r[:], in_=ssq[:],
                                    axis=mybir.AxisListType.X, op=ALU.add)
            usq = const.tile([128, 1], F32, tag=f"usq{idx}")
            nc.scalar.activation(out=usq[:], in_=pad_u[:], func=AF.Square)
            tmp1 = const.tile([128, 1], F32, tag=f"tmp1_{idx}")
            nc.vector.tensor_tensor(out=tmp1[:], in0=pad_u[:], in1=nh_s[:],
                                    op=ALU.mult)
            nc.vector.tensor_tensor(out=sum_r[:], in0=sum_r[:], in1=tmp1[:],
                                    op=ALU.subtract)
            nc.vector.tensor_tensor(out=tmp1[:], in0=usq[:], in1=nh_s[:],
                                    op=ALU.mult)
            nc.vector.tensor_tensor(out=ssq_r[:], in0=ssq_r[:], in1=tmp1[:],
                                    op=ALU.subtract)
            pack = const.tile([128, 2], F32, tag=f"pack{idx}")
            nc.vector.tensor_copy(out=pack[:, 0:1], in_=sum_r[:])
            nc.vector.tensor_copy(out=pack[:, 1:2], in_=ssq_r[:])
            if KDBG:
                nc.sync.dma_start(out=dbg[f"d_pack{idx}"][:], in_=pack[:])
                nc.sync.dma_start(out=dbg[f"d_u{idx}"][:], in_=zu_t[:, :512])
            nc.sync.dma_start(out=ari[:], in_=pack[:])
            ar = nc.gpsimd.collective_compute(
                "AllReduce", ALU.add, replica_groups=rg,
                ins=[ari[:]], outs=[aro[:]])
            rb = const.tile([128, 2], F32, tag=f"rb{idx}")
            d = nc.sync.dma_start(out=rb[:], in_=aro[:])
            add_dep_helper(d.ins, ar.ins, True, "read after AR")
            mean = const.tile([128, 1], F32, tag=f"mean{idx}")
            m2 = const.tile([128, 1], F32, tag=f"m2{idx}")
            nc.scalar.activation(out=mean[:], in_=rb[:, 0:1], func=AF.Copy,
                                 scale=1.0 / c.N)
            nc.scalar.activation(out=m2[:], in_=rb[:, 1:2], func=AF.Copy,
                                 scale=1.0 / c.N)
            msq = const.tile([128, 1], F32, tag=f"msq{idx}")
            nc.scalar.activation(out=msq[:], in_=mean[:], func=AF.Square)
            var = const.tile([128, 1], F32, tag=f"var{idx}")
            nc.vector.tensor_tensor(out=var[:], in0=m2[:], in1=msq[:],
                                    op=ALU.subtract)
            nc.vector.tensor_scalar_add(out=var[:], in0=var[:], scalar1=BN_EPS)
            std = const.tile([128, 1], F32, tag=f"std{idx}")
            nc.scalar.activation(out=std[:], in_=var[:], func=AF.Sqrt)
            inv = const.tile([128, 1], F32, tag=f"inv{idx}")
            nc.vector.reciprocal(out=inv[:], in_=std[:])
            sc = const.tile([128, 1], F32, tag=f"sc{idx}")
            nc.vector.tensor_tensor(out=sc[:], in0=gam_s[:], in1=inv[:],
                                    op=ALU.mult)
            sh = const.tile([128, 1], F32, tag=f"sh{idx}")
            nc.vector.tensor_tensor(out=sh[:], in0=mean[:], in1=sc[:],
                                    op=ALU.mult)
            nc.vector.tensor_tensor(out=sh[:], in0=bet_s[:], in1=sh[:],
                                    op=ALU.subtract)
            if KDBG:
                bnp = const.tile([128, 2], F32, tag=f"bnp{idx}")
                nc.vector.tensor_copy(out=bnp[:, 0:1], in_=sc[:])
                nc.vector.tensor_copy(out=bnp[:, 1:2], in_=sh[:])
                nc.sync.dma_start(out=dbg[f"d_bn{idx}"][:], in_=bnp[:])
            return sc, sh

        cvs = plan["conv"]

        # ---- conv1
        src_views1 = [xg_d[int(r1_lo[q]):int(r1_lo[q + 1]), :]
                      for q in range(c.NR)]
        sc1, sh1 = conv(1, cvs[0], src_views1,
                        ws["c1_w1"], ws["c1_b1"], ws["c1_gamma"], ws["c1_beta"],
                        ws["c1_b1"], ar_in[0], ar_out[0])

        # ---- conv1 tail per slot-half: BN apply, layer2 (feat-major,
        # h1 written in place into zu_t), transposes, h1loc DMA, AllGather
        ag_list = []
        bin_lo = 0
        for q in range(2):
            dmas = []
            for g in range(bin_lo, bin_lo + nbh[q]):
                cols = slice(g * 512, (g + 1) * 512)
                nc.scalar.activation(out=zu_t[:, cols], in_=zu_t[:, cols],
                                     func=AF.Relu, bias=sh1[:, :1],
                                     scale=sc1[:, :1])
                hp = pp.tile([128, 512], F32, tag="up")
                nc.tensor.matmul(out=hp[:], lhsT=ws["c1_w2"][:],
                                 rhs=zu_t[:, cols], start=True, stop=True)
                nc.scalar.activation(out=zu_t[:, cols], in_=hp[:],
                                     func=AF.Relu, bias=ws["c1_b2"][:, :1])
                hnm = work.tile([128, 4 * D], GDT, tag="hnm")
                for t in range(4):
                    tp = pp.tile([128, 128], F32, tag="tp")
                    nc.tensor.transpose(
                        out=tp[:], in_=zu_t[:, g * 512 + t * 128:
                                            g * 512 + (t + 1) * 128],
                        identity=ident_s[:])
                    nc.vector.tensor_copy(out=hnm[:, t * D:(t + 1) * D],
                                          in_=tp[:])
                d = nc.sync.dma_start(
                    out=h1loc_d[g * 512:(g + 1) * 512, :].rearrange(
                        "(b p) f -> p b f", p=128),
                    in_=hnm[:].rearrange("p (b f) -> p b f", b=4))
                dmas.append(d)
            ag = nc.gpsimd.collective_compute(
                "AllGather", ALU.bypass, replica_groups=rg,
                ins=[h1loc_d[int(hslot_lo[q]):int(hslot_lo[q + 1]), :]],
                outs=[h1all_d[q][:]])
            for d in dmas:
                add_dep_helper(ag.ins, d.ins, True, "AG after h1loc")
            ag_list.append(ag)
            bin_lo += nbh[q]
        if KDBG:
            nc.sync.dma_start(out=dbg["d_h1"][:], in_=zu_t[:, :512])

        # pad-column layer1 value for conv2 stats correction:
        # q = c2_w1^T relu(c1_w2^T relu(sc1*c1_b1 + sh1) + c1_b2) + c2_b1
        cvec = const.tile([128, 1], F32, tag="cvec")
        nc.scalar.activation(out=cvec[:], in_=ws["c1_b1"][:], func=AF.Relu,
                             bias=sh1[:, :1], scale=sc1[:, :1])
        pv = pp.tile([128, 1], F32, tag="tp")
        nc.tensor.matmul(out=pv[:], lhsT=ws["c1_w2"][:], rhs=cvec[:],
                         start=True, stop=True)
        pvec = const.tile([128, 1], F32, tag="pvec")
        nc.scalar.activation(out=pvec[:], in_=pv[:], func=AF.Relu,
                             bias=ws["c1_b2"][:, :1])
        qv = pp.tile([128, 1], F32, tag="tp")
        nc.tensor.matmul(out=qv[:], lhsT=ws["c2_w1"][:], rhs=pvec[:],
                         start=True, stop=True)
        qvec = const.tile([128, 1], F32, tag="qvec")
        nc.scalar.activation(out=qvec[:], in_=qv[:], func=AF.Identity,
                             bias=ws["c2_b1"][:, :1])

        # ---- conv2 (gathers wait per-pass on the matching AllGather)
        src_views2 = [
            h1all_d[p // 2][(p % 2) * 4 * int(Hsz[p // 2]):
                            (p % 2 + 1) * 4 * int(Hsz[p // 2]), :]
            for p in range(c.NR)]
        sc2, sh2 = conv(2, cvs[1], src_views2,
                        ws["c2_w1"], ws["c2_b1"], ws["c2_gamma"], ws["c2_beta"],
                        qvec, ar_in[1], ar_out[1],
                        pass_deps=[ag_list[0], ag_list[0],
                                   ag_list[1], ag_list[1]])

        # ---- conv2 tail: BN apply + node-major layer2 + pooling per bin
        b2bp = pp.tile([128, 128], F32, tag="tp")
        nc.tensor.matmul(out=b2bp[:], lhsT=ones_row[:], rhs=b2r_s[:],
                         start=True, stop=True)
        b2bc = const.tile([128, 128], F32, tag="b2bc")
        nc.vector.tensor_copy(out=b2bc[:], in_=b2bp[:])
        for g in range(nbin):
            cols = slice(g * 512, (g + 1) * 512)
            nc.scalar.activation(out=zu_t[:, cols], in_=zu_t[:, cols],
                                 func=AF.Relu, bias=sh2[:, :1],
                                 scale=sc2[:, :1])
            for t in range(4):
                k = g * 4 + t
                hpT = pp.tile([128, 128], F32, tag="tp")
                nc.tensor.matmul(
                    out=hpT[:],
                    lhsT=zu_t[:, g * 512 + t * 128:g * 512 + (t + 1) * 128],
                    rhs=ws["c2_w2"][:], start=True, stop=True,
                    skip_group_check=True)
                hTp = work.tile([128, 128], F32, tag="hTp")
                nc.vector.tensor_tensor(out=hTp[:], in0=hpT[:], in1=b2bc[:],
                                        op=ALU.add)
                hT = work.tile([128, 128], F32, tag="hT")
                nc.scalar.activation(out=hT[:], in_=hTp[:], func=AF.Relu)
                lo = int(win_lo[k])
                poolw = pp.tile([128, c.GW], F32, tag="up")
                nc.tensor.matmul(out=poolw[:], lhsT=hT[:],
                                 rhs=pmat_s[:, k * c.GW:(k + 1) * c.GW],
                                 start=True, stop=True)
                nc.vector.tensor_tensor(
                    out=pooled_acc[:, lo:lo + c.GW],
                    in0=pooled_acc[:, lo:lo + c.GW],
                    in1=poolw[:], op=ALU.add)

        if KDBG:
            nc.sync.dma_start(out=dbg["d_pooled"][:], in_=pooled_acc[:])

        # =========================== head
        hd1 = pp3.tile([128, GPC], F32, tag="zp")
        nc.tensor.matmul(out=hd1[:], lhsT=ws["g_l1_w"][:], rhs=pooled_acc[:],
                         start=True, stop=True)
        t_s = const.tile([128, GPC], F32, tag="t_s")
        nc.scalar.activation(out=t_s[:], in_=hd1[:], func=AF.Relu,
                             bias=ws["g_l1_b"][:, :1])
        hd2 = pp.tile([OUT, GPC], F32, tag="up")
        nc.tensor.matmul(out=hd2[:], lhsT=ws["g_l2_w"][:], rhs=t_s[:],
                         start=True, stop=True)
        trans_embT = const.tile([OUT, GPC], F32, tag="trans_embT")
        nc.scalar.activation(out=trans_embT[:], in_=hd2[:], func=AF.Identity,
                             bias=ws["g_l2_b"][:, :1])
        fp = pp.tile([FIN, GPC], F32, tag="tp")
        nc.tensor.matmul(out=fp[:], lhsT=ws["fin_w"][0:OUT, :],
                         rhs=code_embT[:], start=True, stop=False,
                         skip_group_check=True)
        nc.tensor.matmul(out=fp[:], lhsT=finw_hi[:],
                         rhs=trans_embT[:], start=False, stop=True,
                         skip_group_check=True)
        f_s = const.tile([FIN, GPC], F32, tag="f_s")
        nc.scalar.activation(out=f_s[:], in_=fp[:], func=AF.Identity,
                             bias=ws["fin_b"][:, :1])
        ef = const.tile([FIN, GPC], F32, tag="ef")
        nc.scalar.activation(out=ef[:], in_=f_s[:], func=AF.Exp)
        lfp = pp.tile([1, GPC], F32, tag="up")
        nc.tensor.matmul(out=lfp[:], lhsT=ones_f1[:], rhs=ef[:],
                         start=True, stop=True)
        lf_s = const.tile([1, GPC], F32, tag="lf_s")
        nc.scalar.activation(out=lf_s[:], in_=lfp[:], func=AF.Ln)
        bfp = pp3.tile([FIN, GPC], F32, tag="zp")
        nc.tensor.matmul(out=bfp[:], lhsT=ones_1f[:], rhs=lf_s[:],
                         start=True, stop=True)
        outT = const.tile([FIN, GPC], F32, tag="outT")
        nc.vector.tensor_tensor(out=outT[:], in0=f_s[:], in1=bfp[:],
                                op=ALU.subtract)
        nc.sync.dma_start(out=out_d[:], in_=outT[:])

    if not nc.is_finalized():
        nc.finalize()
    return nc


# ---------------------------------------------------------------- runner

def make_in_maps(inputs, plan, cfg):
    c = cfg
    wnames = ["c1_w1", "c1_b1", "c1_gamma", "c1_beta", "c1_w2", "c1_b2",
              "c2_w1", "c2_b1", "c2_gamma", "c2_beta", "c2_w2", "c2_b2",
              "g_l1_w", "g_l1_b", "g_l2_w", "g_l2_b",
              "fc1_w", "fc1_b", "fc2_w", "fc2_b", "fc3_w", "fc3_b",
              "fin_w", "fin_b"]
    np_gdt = np.float32 if c.GDT == F32 else __import__("ml_dtypes").bfloat16
    x = np.asarray(inputs["x"], np.float32)
    xg = x.astype(np_gdt)
    S = plan["S"]
    node_lo = plan["node_lo"]
    code = np.ascontiguousarray(np.asarray(inputs["code_x"], np.float32))
    ident = np.eye(128, dtype=np.float32)
    b2r = np.ascontiguousarray(
        np.asarray(inputs["c2_b2"], np.float32).reshape(1, c.D))
    in_maps = []
    for ci in range(c.W):
        lo, hi = int(node_lo[ci]), int(node_lo[ci + 1])
        xownT = np.zeros((128, S), np.float32)
        xownT[:, :hi - lo] = x[lo:hi].T
        m = {
            "xg": xg,
            "xownT": xownT,
            "pmat": plan["pmats"][ci],
            "code": code[ci * c.GPC:(ci + 1) * c.GPC],
            "ident": ident,
            "nh": np.full((128, 1), float(S - plan["n_real"][ci]), np.float32),
            "c2_b2r": b2r,
        }
        for li in (0, 1):
            cv = plan["conv"][li]
            m[f"idx{li}"] = cv["cores"][ci]["idx16"]
            m[f"col{li}"] = cv["cores"][ci]["colidx"].astype(np_gdt)
        for k in wnames:
            m[k] = np.ascontiguousarray(np.asarray(inputs[k], np.float32))
        in_maps.append(m)
    return in_maps


_CACHE = {}


def _get_compiled(inputs, cfg):
    if "prog" not in _CACHE:
        plan = _plan(inputs["edge_index"], inputs["batch"], cfg)
        nc = _build(plan, cfg)
        _CACHE["prog"] = (plan, nc)
    return _CACHE["prog"]


def kernel(**inputs) -> np.ndarray:
    cfg = DEFAULT_CFG
    plan, nc = _get_compiled(inputs, cfg)
    in_maps = make_in_maps(inputs, plan, cfg)
    res = run_bass_kernel_spmd(nc, in_maps, core_ids=list(range(cfg.W)))
    outs = [res.results[ci]["out"].T for ci in range(cfg.W)]
    return np.ascontiguousarray(np.concatenate(outs, axis=0).astype(np.float32))


# revision 23
# speedup vs baseline: 2.9825x; 2.5018x over previous
"""Trainium2 Bass kernel for the GIN message-passing model (8 NeuronCores).

Sharding: graph partitioning.  Core c owns graphs [c*G/8, (c+1)*G/8) and the
contiguous node range of those graphs (batch is sorted), plus every edge whose
dst lands there.  dst nodes get compact slot ranks.

Aggregation: edges gathered with `dma_gather` (int16 indices => source row
space split into 4 contiguous ranges = 4 passes).  Gathers cycle across 4
SWDGE queues (ops of <=2048 idxs) so SDMA descriptor drains overlap ~4x.
Within a pass edges are dst-sorted and packed into 128-position chunks
aligned to 128-slot subbins; a chunk's segment-sum is one matmul (gathered
rows stationary, on-chip one-hot moving) into the subbin's slice of a
512-slot PSUM bin.  Self-edges are NOT gathered: z is pre-initialized with
the node's own features (host-transposed x for conv1; in-place h1 for conv2).

conv layer-1 matmuls are interleaved per-bin into the chunk stream so they
overlap the gather tail.  conv2's 4 source ranges are slot-quarters of each
core's h1, published by 4 pipelined AllGathers so conv2 pass-q gathers start
as soon as AG_q lands.  conv2's layer-2 runs node-major (no transposes),
feeding pooling matmuls directly; BN pad corrections are computed on device.
"""

import os
import sys

for _p in ("/opt/trn_rl_repo",):
    if _p not in sys.path:
        sys.path.insert(0, _p)

KDBG = int(os.environ.get("KDBG", "0"))

import numpy as np
from contextlib import ExitStack

import concourse.bass as bass
import concourse.bacc as bacc
import concourse.mybir as mybir
import concourse.tile as tile
from concourse.bass_utils import run_bass_kernel_spmd
from concourse.tile_rust import add_dep_helper

F32 = mybir.dt.float32
BF16 = mybir.dt.bfloat16
I32 = mybir.dt.int32
I16 = mybir.dt.int16
AF = mybir.ActivationFunctionType
ALU = mybir.AluOpType

BN_EPS = 1e-5
PADCOL = 200.0          # colidx value for pad positions (never matches 0..127)


class Cfg:
    def __init__(self, N=100000, E=500000, G=2048, D=128, OUT=64, FIN=2, W=8,
                 NR=4, NIMAX=2048, NQ=4, GW=32, GDT=BF16):
        self.N, self.E, self.G, self.D, self.OUT, self.FIN, self.W = N, E, G, D, OUT, FIN, W
        self.NR = NR        # source ranges (int16 index limit)
        self.NIMAX = NIMAX  # max positions per dma_gather
        self.NQ = NQ        # SWDGE queues to cycle gathers over
        self.GW = GW        # pooling window width (graphs)
        self.GDT = GDT      # gather dtype (bf16 or f32)
        self.GPC = G // W   # graphs per core


DEFAULT_CFG = Cfg()


def _wrap_idx(lst):
    """dma_gather index layout: position j is read from row j%16, col j//16."""
    assert len(lst) % 16 == 0
    return np.tile(np.asarray(lst, np.int16).reshape(-1, 16).T, (8, 1))


# ---------------------------------------------------------------- host plan

def _plan(edge_index, batch, cfg):
    c = cfg
    batch = np.asarray(batch).astype(np.int64)
    ei = np.asarray(edge_index).astype(np.int64)
    owner = (batch // c.GPC).astype(np.int64)

    src, dst = ei[0], ei[1]          # no synthetic self-edges
    eowner = owner[dst]

    # compact slot ranks per core
    n_real = np.zeros(c.W, np.int64)
    slot_of = np.full(c.N, -1, np.int64)
    node_lo = np.zeros(c.W + 1, np.int64)
    for ci in range(c.W):
        node_lo[ci] = np.searchsorted(batch, ci * c.GPC)
    node_lo[c.W] = c.N
    for ci in range(c.W):
        lo, hi = node_lo[ci], node_lo[ci + 1]
        n_real[ci] = hi - lo
        slot_of[lo:hi] = np.arange(hi - lo)
    S = int(((n_real.max() + 511) // 512) * 512)
    nbin = S // 512
    nsub = S // 128
    nSC = S // 128

    # conv1 source ranges: contiguous quarters of the x row space
    r1_lo = np.array([(c.N * q) // c.NR for q in range(c.NR + 1)], np.int64)
    assert (np.diff(r1_lo) <= 32767).all()
    epass1 = np.searchsorted(r1_lo[1:], src, side="right")
    loc1 = src - r1_lo[epass1]

    # conv2 source ranges: (slot-half, core-group-of-4).  Each half of the
    # slot space is published by one AllGather; within a half the 4 ranges
    # split by core group so every range stays under the int16 limit.
    nbh = [(nbin + 1) // 2, nbin // 2]
    hslot_lo = np.array([0, nbh[0] * 512, S], np.int64)
    Hsz = np.diff(hslot_lo)
    assert (4 * Hsz <= 32767).all()
    s_src = slot_of[src]
    o_src = owner[src]
    half = (s_src >= hslot_lo[1]).astype(np.int64)
    epass2 = 2 * half + o_src // 4
    loc2 = (o_src % 4) * Hsz[half] + (s_src - hslot_lo[half])

    def build_conv(epass, loc, range_rows):
        """epass: per-edge source range id; loc: row within that range.
        range_rows: rows per source range (pad positions are spread across
        the range — clustering them on row 0 serializes one HBM channel).
        Returns common chunk structure + per-core idx/colidx arrays."""
        counts = np.zeros((c.W, c.NR, nsub), np.int64)
        percore_edges = []
        for ci in range(c.W):
            m = eowner == ci
            sl = slot_of[dst[m]]
            pr = epass[m]
            rows = loc[m]
            sub = sl // 128
            order = np.lexsort((sl, sub, pr))
            sl, pr, rows, sub = sl[order], pr[order], rows[order], sub[order]
            np.add.at(counts[ci], (pr, sub), 1)
            percore_edges.append((sl, pr, rows, sub))
        # common chunk structure (max over cores per segment).  Even empty
        # segments get one (all-pad) chunk: its matmul zero-initializes the
        # PSUM slice that the per-bin fold reads (else stale PSUM garbage
        # lands in pad slots and poisons the BN stats).
        nch = np.maximum(1, -(-counts.max(axis=0) // 128))  # [NR, nsub]
        chunks = []     # (pass, subbin)
        for r in range(c.NR):
            for sb in range(nsub):
                for k in range(int(nch[r, sb])):
                    chunks.append((r, sb))
        C = len(chunks)
        POS = C * 128
        pass_pos_lo = np.zeros(c.NR + 1, np.int64)
        for r in range(c.NR):
            pass_pos_lo[r + 1] = pass_pos_lo[r] + 128 * int(nch[r].sum())
        # per-core arrays
        cores = []
        pad_idx = np.zeros(POS, np.int64)
        for r in range(c.NR):
            plo, phi = int(pass_pos_lo[r]), int(pass_pos_lo[r + 1])
            pad_idx[plo:phi] = (np.arange(phi - plo) * 97) % int(range_rows[r])
        for ci in range(c.W):
            sl, pr, rows, sub = percore_edges[ci]
            idx_local = pad_idx.copy()
            colv = np.full((128, C), PADCOL, np.float64)
            seg_base = {}
            pos = 0
            for r in range(c.NR):
                for sb in range(nsub):
                    seg_base[(r, sb)] = pos
                    pos += 128 * int(nch[r, sb])
            key = pr * nsub + sub
            uniq, start_idx = np.unique(key, return_index=True)
            end_idx = np.append(start_idx[1:], len(key))
            for u, s0, s1 in zip(uniq, start_idx, end_idx):
                r, sb = int(u) // nsub, int(u) % nsub
                base_ = seg_base[(r, sb)]
                n = s1 - s0
                p = base_ + np.arange(n)
                idx_local[p] = rows[s0:s1]
                colv[p % 128, p // 128] = sl[s0:s1] - sb * 128
            wrapped = [
                _wrap_idx(idx_local[pass_pos_lo[r]:pass_pos_lo[r + 1]])
                for r in range(c.NR) if pass_pos_lo[r + 1] > pass_pos_lo[r]
            ]
            idx16 = np.concatenate(wrapped, axis=1) if wrapped else np.zeros((128, 0), np.int16)
            cores.append(dict(idx16=idx16, colidx=colv))
        # gather op list: per pass, ops of <= NIMAX positions
        ops = []        # (pass, pos_lo, ni)
        for r in range(c.NR):
            p0, p1 = int(pass_pos_lo[r]), int(pass_pos_lo[r + 1])
            while p0 < p1:
                ni = min(c.NIMAX, p1 - p0)
                ops.append((r, p0, ni))
                p0 += ni
        return dict(C=C, POS=POS, chunks=chunks, ops=ops, cores=cores)

    conv1 = build_conv(epass1, loc1, np.diff(r1_lo))
    conv2 = build_conv(epass2, loc2, [4 * int(Hsz[p // 2]) for p in range(c.NR)])

    # pooling plan
    gos_all = []
    for ci in range(c.W):
        gos = np.full(S, -1, np.int64)
        lo, hi = node_lo[ci], node_lo[ci + 1]
        gos[:hi - lo] = batch[lo:hi] - ci * c.GPC
        gos_all.append(gos)
    win_lo = np.zeros(nSC, np.int64)
    prev = 0
    for k in range(nSC):
        lo_k, hi_k = c.GPC, -1
        for gos in gos_all:
            seg = gos[k * 128:(k + 1) * 128]
            v = seg[seg >= 0]
            if len(v):
                lo_k = min(lo_k, int(v.min()))
                hi_k = max(hi_k, int(v.max()))
        if hi_k < 0:
            lo_k = hi_k = min(prev, c.GPC - 1)
        assert hi_k - lo_k + 1 <= c.GW, f"pool window too wide: {lo_k}..{hi_k}"
        lo_k = max(0, min(lo_k, c.GPC - c.GW))
        assert lo_k <= prev + c.GW, "pool window coverage gap"
        win_lo[k] = lo_k
        prev = max(prev, lo_k + c.GW - 1)
    covered = np.zeros(c.GPC, bool)
    for k in range(nSC):
        covered[win_lo[k]:win_lo[k] + c.GW] = True
    assert covered.all()

    pmats = []
    for ci in range(c.W):
        pmat = np.zeros((128, nSC * c.GW), np.float32)
        gos = gos_all[ci]
        for k in range(nSC):
            seg = gos[k * 128:(k + 1) * 128]
            for p in range(128):
                if seg[p] >= 0:
                    w = int(seg[p] - win_lo[k])
                    pmat[p, k * c.GW + w] = 1.0
        pmats.append(pmat)

    return dict(S=S, nbin=nbin, nSC=nSC, win_lo=win_lo, conv=[conv1, conv2],
                n_real=n_real, node_lo=node_lo, r1_lo=r1_lo,
                hslot_lo=hslot_lo, Hsz=Hsz, nbh=nbh, pmats=pmats)


# ---------------------------------------------------------------- program

def _build(plan, cfg):
    c = cfg
    S, nbin, nSC = plan["S"], plan["nbin"], plan["nSC"]
    win_lo = plan["win_lo"]
    hslot_lo, Hsz, nbh = plan["hslot_lo"], plan["Hsz"], plan["nbh"]
    r1_lo = plan["r1_lo"]
    D, OUT, FIN, GPC = c.D, c.OUT, c.FIN, c.GPC
    rg = [list(range(c.W))]
    nG = nbin
    GDT = c.GDT

    nc = bacc.Bacc(num_devices=c.W, num_swdge_queues=c.NQ)

    # ---- external inputs
    xg_d = nc.dram_tensor("xg", [c.N, D], GDT, kind="ExternalInput")
    xownT_d = nc.dram_tensor("xownT", [128, S], F32, kind="ExternalInput")
    pmat_d = nc.dram_tensor("pmat", [128, nSC * c.GW], F32, kind="ExternalInput")
    idx_d, col_d = [], []
    for li in (0, 1):
        cv = plan["conv"][li]
        idx_d.append(nc.dram_tensor(f"idx{li}", [128, cv["POS"] // 16], I16,
                                    kind="ExternalInput"))
        col_d.append(nc.dram_tensor(f"col{li}", [128, cv["C"]], GDT,
                                    kind="ExternalInput"))
    code_d = nc.dram_tensor("code", [GPC, D], F32, kind="ExternalInput")
    ident_d = nc.dram_tensor("ident", [128, 128], F32, kind="ExternalInput")
    nh_d = nc.dram_tensor("nh", [128, 1], F32, kind="ExternalInput")

    wspec = {
        "c1_w1": [D, D], "c1_b1": [D], "c1_gamma": [D], "c1_beta": [D],
        "c1_w2": [D, D], "c1_b2": [D],
        "c2_w1": [D, D], "c2_b1": [D], "c2_gamma": [D], "c2_beta": [D],
        "c2_w2": [D, D], "c2_b2": [D],
        "g_l1_w": [D, D], "g_l1_b": [D], "g_l2_w": [D, OUT], "g_l2_b": [OUT],
        "fc1_w": [D, D], "fc1_b": [D], "fc2_w": [D, D], "fc2_b": [D],
        "fc3_w": [D, OUT], "fc3_b": [OUT],
        "fin_w": [2 * OUT, FIN], "fin_b": [FIN],
    }
    wd = {k: nc.dram_tensor(k, v, F32, kind="ExternalInput") for k, v in wspec.items()}
    b2r_d = nc.dram_tensor("c2_b2r", [1, D], F32, kind="ExternalInput")

    out_d = nc.dram_tensor("out", [FIN, GPC], F32, kind="ExternalOutput")
    dbg = {}
    if KDBG:
        for nm, shp in [("d_pack1", [128, 2]), ("d_pack2", [128, 2]),
                        ("d_bn1", [128, 2]), ("d_bn2", [128, 2]),
                        ("d_u1", [128, 512]), ("d_u2", [128, 512]),
                        ("d_h1", [128, 512]), ("d_pooled", [128, GPC])]:
            dbg[nm] = nc.dram_tensor(nm, shp, F32, kind="ExternalOutput")

    # ---- internal DRAM
    h1loc_d = nc.dram_tensor("h1loc", [S, D], GDT)
    h1all_d = [nc.dram_tensor(f"h1all{h}", [c.W * int(Hsz[h]), D], GDT,
                              addr_space="Shared") for h in range(2)]
    ar_in = [nc.dram_tensor(f"ar{i}i", [128, 2], F32) for i in (1, 2)]
    ar_out = [nc.dram_tensor(f"ar{i}o", [128, 2], F32, addr_space="Shared")
              for i in (1, 2)]

    with tile.TileContext(nc) as tc, ExitStack() as ctx:
        const = ctx.enter_context(tc.tile_pool(name="const", bufs=1))
        work = ctx.enter_context(tc.tile_pool(name="work", bufs=3))
        gwork = ctx.enter_context(tc.tile_pool(name="gwork", bufs=12))
        swork = ctx.enter_context(tc.tile_pool(name="swork", bufs=6))
        wide = ctx.enter_context(tc.tile_pool(name="wide", bufs=1))
        pp = ctx.enter_context(tc.tile_pool(name="pp", bufs=2, space="PSUM"))
        pp3 = ctx.enter_context(tc.tile_pool(name="pp3", bufs=3, space="PSUM"))

        def cload(dram_ap, shape, dtype, tag):
            t = const.tile(shape, dtype, tag=tag)
            nc.sync.dma_start(out=t[:], in_=dram_ap)
            return t

        # idx/col tables first: the gather stream depends on them and nothing
        # else, so they must not queue behind the bulky weight/const DMAs.
        idxcol = {}
        for li, cv_ in enumerate(plan["conv"]):
            i_s = const.tile([128, cv_["POS"] // 16], I16, tag=f"idx{li+1}")
            nc.sync.dma_start(out=i_s[:], in_=idx_d[li][:])
            c_s = const.tile([128, cv_["C"]], GDT, tag=f"col{li+1}")
            nc.sync.dma_start(out=c_s[:], in_=col_d[li][:])
            idxcol[li + 1] = (i_s, c_s)

        ident_s = cload(ident_d[:], [128, 128], F32, "ident")
        nh_s = cload(nh_d[:], [128, 1], F32, "nh")
        pmat_s = cload(pmat_d[:], [128, nSC * c.GW], F32, "pmat")
        b2r_s = cload(b2r_d[:], [1, D], F32, "b2r")

        ws = {}
        for k, shp in wspec.items():
            if len(shp) == 2:
                ws[k] = cload(wd[k][:], shp, F32, k)
            else:
                ws[k] = cload(wd[k][:, None], [shp[0], 1], F32, k)
        finw_hi = const.tile([OUT, FIN], F32, tag="finw_hi")
        nc.sync.dma_start(out=finw_hi[:], in_=wd["fin_w"][OUT:2 * OUT, :])

        # iota row pattern repeated (for one-hot gen), in gather dtype
        IOB = 8  # chunks per one-hot op
        iota_i = const.tile([128, IOB * 128], I32, tag="iota_i")
        nc.gpsimd.iota(iota_i[:], pattern=[[0, IOB], [1, 128]], base=0,
                       channel_multiplier=0)
        iota_s = const.tile([128, IOB * 128], GDT, tag="iota_s")
        nc.vector.tensor_copy(out=iota_s[:], in_=iota_i[:])

        ones_d1 = const.tile([OUT, 1], F32, tag="ones_d1")
        nc.vector.memset(ones_d1[:], 1.0)
        ones_1d = const.tile([1, OUT], F32, tag="ones_1d")
        nc.vector.memset(ones_1d[:], 1.0)
        ones_f1 = const.tile([FIN, 1], F32, tag="ones_f1")
        nc.vector.memset(ones_f1[:], 1.0)
        ones_1f = const.tile([1, FIN], F32, tag="ones_1f")
        nc.vector.memset(ones_1f[:], 1.0)
        ones_row = const.tile([1, 128], F32, tag="ones_row")
        nc.vector.memset(ones_row[:], 1.0)

        # z accumulator, pre-initialized with own-node features (self term)
        zu_t = wide.tile([128, S], F32, tag="zu")
        nc.sync.dma_start(out=zu_t[:], in_=xownT_d[:])
        pooled_acc = const.tile([128, GPC], F32, tag="pooled_acc")
        nc.vector.memset(pooled_acc[:], 0.0)

        # =========================== code MLP branch (fills bubbles)
        nbl = (GPC + 127) // 128
        code_nm = const.tile([128, nbl * D], F32, tag="code_nm")
        nc.sync.dma_start(
            out=code_nm[:].rearrange("p (b f) -> p b f", b=nbl),
            in_=code_d[:].rearrange("(b p) f -> p b f", p=128))
        codeT = const.tile([128, GPC], F32, tag="codeT")
        for b in range(nbl):
            tp = pp.tile([128, 128], F32, tag="tp")
            nc.tensor.transpose(out=tp[:], in_=code_nm[:, b * D:(b + 1) * D],
                                identity=ident_s[:])
            nc.vector.tensor_copy(out=codeT[:, b * 128:(b + 1) * 128], in_=tp[:])
        cps = pp3.tile([128, GPC], F32, tag="zp")
        nc.tensor.matmul(out=cps[:], lhsT=ws["fc1_w"][:], rhs=codeT[:],
                         start=True, stop=True)
        c1_s = const.tile([128, GPC], F32, tag="c1_s")
        nc.scalar.activation(out=c1_s[:], in_=cps[:], func=AF.Relu,
                             bias=ws["fc1_b"][:, :1])
        cps2 = pp3.tile([128, GPC], F32, tag="zp")
        nc.tensor.matmul(out=cps2[:], lhsT=ws["fc2_w"][:], rhs=c1_s[:],
                         start=True, stop=True)
        c2_s = const.tile([128, GPC], F32, tag="c2_s")
        nc.scalar.activation(out=c2_s[:], in_=cps2[:], func=AF.Relu,
                             bias=ws["fc2_b"][:, :1])
        cps3 = pp.tile([OUT, GPC], F32, tag="up")
        nc.tensor.matmul(out=cps3[:], lhsT=ws["fc3_w"][:], rhs=c2_s[:],
                         start=True, stop=True)
        c3_s = const.tile([OUT, GPC], F32, tag="c3_s")
        nc.scalar.activation(out=c3_s[:], in_=cps3[:], func=AF.Identity,
                             bias=ws["fc3_b"][:, :1])
        e64 = const.tile([OUT, GPC], F32, tag="e64")
        nc.scalar.activation(out=e64[:], in_=c3_s[:], func=AF.Exp)
        lsp = pp.tile([1, GPC], F32, tag="tp")
        nc.tensor.matmul(out=lsp[:], lhsT=ones_d1[:], rhs=e64[:],
                         start=True, stop=True)
        lse_s = const.tile([1, GPC], F32, tag="lse_s")
        nc.scalar.activation(out=lse_s[:], in_=lsp[:], func=AF.Ln)
        bcp = pp.tile([OUT, GPC], F32, tag="up")
        nc.tensor.matmul(out=bcp[:], lhsT=ones_1d[:], rhs=lse_s[:],
                         start=True, stop=True)
        code_embT = const.tile([OUT, GPC], F32, tag="code_embT")
        nc.vector.tensor_tensor(out=code_embT[:], in0=c3_s[:], in1=bcp[:],
                                op=ALU.subtract)

        # =========================== GIN convs
        def conv(idx, cv, src_views, w1_s, b1_s, gam_s, bet_s,
                 pad_u, ari, aro, pass_deps=None):
            """Chunk loop with interleaved per-bin layer1, then BN stats +
            AllReduce + BN params.  Returns (sc, sh) tiles.
            src_views: per-pass DRAM APs.  pad_u: [128,1] expected layer1
            value of pad columns (subtracted npad times from the stats).
            pass_deps: per-pass instruction the first gather must wait on."""
            C, POS = cv["C"], cv["POS"]
            chunks, ops = cv["chunks"], cv["ops"]
            idx_s, col_s = idxcol[idx]
            ssum = const.tile([128, nG], F32, tag=f"ssum{idx}")
            ssq = const.tile([128, nG], F32, tag=f"ssq{idx}")

            # map chunk -> (op index, block within op)
            chunk_op = []
            for oi, (r, plo, ni) in enumerate(ops):
                for b in range(ni // 128):
                    chunk_op.append((oi, b))
            assert len(chunk_op) == C

            # per bin: the (bin, pass) group whose close should emit layer1
            last_group_of_bin = {}
            for ci_, (r, sb) in enumerate(chunks):
                last_group_of_bin[sb // 4] = (sb // 4, r)

            def layer1(g):
                cols = slice(g * 512, (g + 1) * 512)
                up = pp.tile([128, 512], F32, tag="up")
                nc.tensor.matmul(out=up[:], lhsT=w1_s[:], rhs=zu_t[:, cols],
                                 start=True, stop=True)
                nc.scalar.activation(out=zu_t[:, cols], in_=up[:],
                                     func=AF.Identity, bias=b1_s[:, :1],
                                     accum_out=ssum[:, g:g + 1])
                sq = work.tile([128, 512], F32, tag="sq")
                nc.scalar.activation(out=sq[:], in_=zu_t[:, cols],
                                     func=AF.Square,
                                     accum_out=ssq[:, g:g + 1])

            gtiles = {}
            stiles = {}
            cur_group = None       # (bin, pass)
            zp = None
            first_op_of_pass = {}
            for oi, (r, plo, ni) in enumerate(ops):
                if r not in first_op_of_pass:
                    first_op_of_pass[r] = oi

            def close_group():
                nonlocal cur_group, zp
                if cur_group is None:
                    return
                bn = cur_group[0]
                cols = slice(bn * 512, (bn + 1) * 512)
                nc.vector.tensor_tensor(out=zu_t[:, cols], in0=zu_t[:, cols],
                                        in1=zp[:], op=ALU.add)
                if last_group_of_bin.get(bn) == cur_group:
                    layer1(bn)
                cur_group, zp = None, None

            for ci in range(C):
                r, sb = chunks[ci]
                bn, sl4 = sb // 4, sb % 4
                oi, blk = chunk_op[ci]
                if oi not in gtiles:
                    opr, plo, ni = ops[oi]
                    gt = gwork.tile([128, c.NIMAX], GDT, tag="gt")
                    g_ins = nc.gpsimd.dma_gather(
                        gt[:, :ni].rearrange("p (k f) -> p k f", k=ni // 128),
                        src_views[opr],
                        idx_s[:, plo // 16:(plo + ni) // 16],
                        ni, ni, 128, elem_step=D,
                        single_packet=False, queue_num=oi % c.NQ)
                    if pass_deps is not None and oi == first_op_of_pass[opr]:
                        add_dep_helper(g_ins.ins, pass_deps[opr].ins, True,
                                       "gather after AG")
                    gtiles = {oi: gt}
                if ci % IOB == 0:
                    nob = min(IOB, C - ci)
                    st = swork.tile([128, IOB * 128], GDT, tag="st")
                    nc.vector.tensor_tensor(
                        out=st[:, :nob * 128].rearrange("p (c f) -> p c f", c=nob),
                        in0=col_s[:, ci:ci + nob].to_broadcast([128, nob, 128]),
                        in1=iota_s[:, :nob * 128].rearrange("p (c f) -> p c f", c=nob),
                        op=ALU.is_equal)
                    stiles = {ci // IOB: st}
                if cur_group != (bn, r):
                    close_group()
                    cur_group = (bn, r)
                    zp = pp3.tile([128, 512], F32, tag="zp")
                is_first = (ci == 0 or chunks[ci - 1][0] != r
                            or chunks[ci - 1][1] // 4 != bn)
                is_last = (ci == C - 1 or chunks[ci + 1][0] != chunks[ci][0]
                           or chunks[ci + 1][1] // 4 != bn)
                nc.tensor.matmul(
                    out=zp[:, sl4 * 128:(sl4 + 1) * 128],
                    lhsT=gtiles[oi][:, blk * 128:(blk + 1) * 128],
                    rhs=stiles[ci // IOB][:, (ci % IOB) * 128:(ci % IOB + 1) * 128],
                    start=is_first, stop=is_last,
                    skip_group_check=True)
            close_group()

            # ---- BN stats + AllReduce
            sum_r = const.tile([128, 1], F32, tag=f"sum_r{idx}")
            ssq_r = const.tile([128, 1], F32, tag=f"ssq_r{idx}")
            nc.vector.tensor_reduce(out=sum_r[:], in_=ssum[:],
                                    axis=mybir.AxisListType.X, op=ALU.add)
            nc.vector.tensor_reduce(out=ssq_r[:], in_=ssq[:],
                                    axis=mybir.AxisListType.X, op=ALU.add)
            usq = const.tile([128, 1], F32, tag=f"usq{idx}")
            nc.scalar.activation(out=usq[:], in_=pad_u[:], func=AF.Square)
            tmp1 = const.tile([128, 1], F32, tag=f"tmp1_{idx}")
            nc.vector.tensor_tensor(out=tmp1[:], in0=pad_u[:], in1=nh_s[:],
                                    op=ALU.mult)
            nc.vector.tensor_tensor(out=sum_r[:], in0=sum_r[:], in1=tmp1[:],
                                    op=ALU.subtract)
            nc.vector.tensor_tensor(out=tmp1[:], in0=usq[:], in1=nh_s[:],
                                    op=ALU.mult)
            nc.vector.tensor_tensor(out=ssq_r[:], in0=ssq_r[:], in1=tmp1[:],
                                    op=ALU.subtract)
            pack = const.tile([128, 2], F32, tag=f"pack{idx}")
            nc.vector.tensor_copy(out=pack[:, 0:1], in_=sum_r[:])
            nc.vector.tensor_copy(out=pack[:, 1:2], in_=ssq_r[:])
            if KDBG:
                nc.sync.dma_start(out=dbg[f"d_pack{idx}"][:], in_=pack[:])
                nc.sync.dma_start(out=dbg[f"d_u{idx}"][:], in_=zu_t[:, :512])
            nc.sync.dma_start(out=ari[:], in_=pack[:])
            ar = nc.gpsimd.collective_compute(
                "AllReduce", ALU.add, replica_groups=rg,
                ins=[ari[:]], outs=[aro[:]])
            rb = const.tile([128, 2], F32, tag=f"rb{idx}")
            d = nc.sync.dma_start(out=rb[:], in_=aro[:])
            add_dep_helper(d.ins, ar.ins, True, "read after AR")
            mean = const.tile([128, 1], F32, tag=f"mean{idx}")
            m2 = const.tile([128, 1], F32, tag=f"m2{idx}")
            nc.scalar.activation(out=mean[:], in_=rb[:, 0:1], func=AF.Copy,
                                 scale=1.0 / c.N)
            nc.scalar.activation(out=m2[:], in_=rb[:, 1:2], func=AF.Copy,
                                 scale=1.0 / c.N)
            msq = const.tile([128, 1], F32, tag=f"msq{idx}")
            nc.scalar.activation(out=msq[:], in_=mean[:], func=AF.Square)
            var = const.tile([128, 1], F32, tag=f"var{idx}")
            nc.vector.tensor_tensor(out=var[:], in0=m2[:], in1=msq[:],
                                    op=ALU.subtract)
            nc.vector.tensor_scalar_add(out=var[:], in0=var[:], scalar1=BN_EPS)
            std = const.tile([128, 1], F32, tag=f"std{idx}")
            nc.scalar.activation(out=std[:], in_=var[:], func=AF.Sqrt)
            inv = const.tile([128, 1], F32, tag=f"inv{idx}")
            nc.vector.reciprocal(out=inv[:], in_=std[:])
            sc = const.tile([128, 1], F32, tag=f"sc{idx}")
            nc.vector.tensor_tensor(out=sc[:], in0=gam_s[:], in1=inv[:],
                                    op=ALU.mult)
            sh = const.tile([128, 1], F32, tag=f"sh{idx}")
            nc.vector.tensor_tensor(out=sh[:], in0=mean[:], in1=sc[:],
                                    op=ALU.mult)
            nc.vector.tensor_tensor(out=sh[:], in0=bet_s[:], in1=sh[:],
                                    op=ALU.subtract)
            if KDBG:
                bnp = const.tile([128, 2], F32, tag=f"bnp{idx}")
                nc.vector.tensor_copy(out=bnp[:, 0:1], in_=sc[:])
                nc.vector.tensor_copy(out=bnp[:, 1:2], in_=sh[:])
                nc.sync.dma_start(out=dbg[f"d_bn{idx}"][:], in_=bnp[:])
            return sc, sh

        cvs = plan["conv"]

        # ---- conv1
        src_views1 = [xg_d[int(r1_lo[q]):int(r1_lo[q + 1]), :]
                      for q in range(c.NR)]
        sc1, sh1 = conv(1, cvs[0], src_views1,
                        ws["c1_w1"], ws["c1_b1"], ws["c1_gamma"], ws["c1_beta"],
                        ws["c1_b1"], ar_in[0], ar_out[0])

        # ---- conv1 tail per slot-half: BN apply, layer2 (feat-major,
        # h1 written in place into zu_t), transposes, h1loc DMA, AllGather
        ag_list = []
        bin_lo = 0
        for q in range(2):
            dmas = []
            for g in range(bin_lo, bin_lo + nbh[q]):
                cols = slice(g * 512, (g + 1) * 512)
                nc.scalar.activation(out=zu_t[:, cols], in_=zu_t[:, cols],
                                     func=AF.Relu, bias=sh1[:, :1],
                                     scale=sc1[:, :1])
                hp = pp.tile([128, 512], F32, tag="up")
                nc.tensor.matmul(out=hp[:], lhsT=ws["c1_w2"][:],
                                 rhs=zu_t[:, cols], start=True, stop=True)
                nc.scalar.activation(out=zu_t[:, cols], in_=hp[:],
                                     func=AF.Relu, bias=ws["c1_b2"][:, :1])
                hnm = work.tile([128, 4 * D], GDT, tag="hnm")
                for t in range(4):
                    tp = pp.tile([128, 128], F32, tag="tp")
                    nc.tensor.transpose(
                        out=tp[:], in_=zu_t[:, g * 512 + t * 128:
                                            g * 512 + (t + 1) * 128],
                        identity=ident_s[:])
                    nc.vector.tensor_copy(out=hnm[:, t * D:(t + 1) * D],
                                          in_=tp[:])
                d = nc.sync.dma_start(
                    out=h1loc_d[g * 512:(g + 1) * 512, :].rearrange(
                        "(b p) f -> p b f", p=128),
                    in_=hnm[:].rearrange("p (b f) -> p b f", b=4))
                dmas.append(d)
            ag = nc.gpsimd.collective_compute(
                "AllGather", ALU.bypass, replica_groups=rg,
                ins=[h1loc_d[int(hslot_lo[q]):int(hslot_lo[q + 1]), :]],
                outs=[h1all_d[q][:]])
            for d in dmas:
                add_dep_helper(ag.ins, d.ins, True, "AG after h1loc")
            ag_list.append(ag)
            bin_lo += nbh[q]
        if KDBG:
            nc.sync.dma_start(out=dbg["d_h1"][:], in_=zu_t[:, :512])

        # pad-column layer1 value for conv2 stats correction:
        # q = c2_w1^T relu(c1_w2^T relu(sc1*c1_b1 + sh1) + c1_b2) + c2_b1
        cvec = const.tile([128, 1], F32, tag="cvec")
        nc.scalar.activation(out=cvec[:], in_=ws["c1_b1"][:], func=AF.Relu,
                             bias=sh1[:, :1], scale=sc1[:, :1])
        pv = pp.tile([128, 1], F32, tag="tp")
        nc.tensor.matmul(out=pv[:], lhsT=ws["c1_w2"][:], rhs=cvec[:],
                         start=True, stop=True)
        pvec = const.tile([128, 1], F32, tag="pvec")
        nc.scalar.activation(out=pvec[:], in_=pv[:], func=AF.Relu,
                             bias=ws["c1_b2"][:, :1])
        qv = pp.tile([128, 1], F32, tag="tp")
        nc.tensor.matmul(out=qv[:], lhsT=ws["c2_w1"][:], rhs=pvec[:],
                         start=True, stop=True)
        qvec = const.tile([128, 1], F32, tag="qvec")
        nc.scalar.activation(out=qvec[:], in_=qv[:], func=AF.Identity,
                             bias=ws["c2_b1"][:, :1])

        # ---- conv2 (gathers wait per-pass on the matching AllGather)
        src_views2 = [
            h1all_d[p // 2][(p % 2) * 4 * int(Hsz[p // 2]):
                            (p % 2 + 1) * 4 * int(Hsz[p // 2]), :]
            for p in range(c.NR)]
        sc2, sh2 = conv(2, cvs[1], src_views2,
                        ws["c2_w1"], ws["c2_b1"], ws["c2_gamma"], ws["c2_beta"],
                        qvec, ar_in[1], ar_out[1],
                        pass_deps=[ag_list[0], ag_list[0],
                                   ag_list[1], ag_list[1]])

        # ---- conv2 tail: BN apply + node-major layer2 + pooling per bin
        b2bp = pp.tile([128, 128], F32, tag="tp")
        nc.tensor.matmul(out=b2bp[:], lhsT=ones_row[:], rhs=b2r_s[:],
                         start=True, stop=True)
        b2bc = const.tile([128, 128], F32, tag="b2bc")
        nc.vector.tensor_copy(out=b2bc[:], in_=b2bp[:])
        for g in range(nbin):
            cols = slice(g * 512, (g + 1) * 512)
            nc.scalar.activation(out=zu_t[:, cols], in_=zu_t[:, cols],
                                 func=AF.Relu, bias=sh2[:, :1],
                                 scale=sc2[:, :1])
            for t in range(4):
                k = g * 4 + t
                hpT = pp.tile([128, 128], F32, tag="tp")
                nc.tensor.matmul(
                    out=hpT[:],
                    lhsT=zu_t[:, g * 512 + t * 128:g * 512 + (t + 1) * 128],
                    rhs=ws["c2_w2"][:], start=True, stop=True,
                    skip_group_check=True)
                hTp = work.tile([128, 128], F32, tag="hTp")
                nc.vector.tensor_tensor(out=hTp[:], in0=hpT[:], in1=b2bc[:],
                                        op=ALU.add)
                hT = work.tile([128, 128], F32, tag="hT")
                nc.scalar.activation(out=hT[:], in_=hTp[:], func=AF.Relu)
                lo = int(win_lo[k])
                poolw = pp.tile([128, c.GW], F32, tag="up")
                nc.tensor.matmul(out=poolw[:], lhsT=hT[:],
                                 rhs=pmat_s[:, k * c.GW:(k + 1) * c.GW],
                                 start=True, stop=True)
                nc.vector.tensor_tensor(
                    out=pooled_acc[:, lo:lo + c.GW],
                    in0=pooled_acc[:, lo:lo + c.GW],
                    in1=poolw[:], op=ALU.add)

        if KDBG:
            nc.sync.dma_start(out=dbg["d_pooled"][:], in_=pooled_acc[:])

        # =========================== head
        hd1 = pp3.tile([128, GPC], F32, tag="zp")
        nc.tensor.matmul(out=hd1[:], lhsT=ws["g_l1_w"][:], rhs=pooled_acc[:],
                         start=True, stop=True)
        t_s = const.tile([128, GPC], F32, tag="t_s")
        nc.scalar.activation(out=t_s[:], in_=hd1[:], func=AF.Relu,
                             bias=ws["g_l1_b"][:, :1])
        hd2 = pp.tile([OUT, GPC], F32, tag="up")
        nc.tensor.matmul(out=hd2[:], lhsT=ws["g_l2_w"][:], rhs=t_s[:],
                         start=True, stop=True)
        trans_embT = const.tile([OUT, GPC], F32, tag="trans_embT")
        nc.scalar.activation(out=trans_embT[:], in_=hd2[:], func=AF.Identity,
                             bias=ws["g_l2_b"][:, :1])
        fp = pp.tile([FIN, GPC], F32, tag="tp")
        nc.tensor.matmul(out=fp[:], lhsT=ws["fin_w"][0:OUT, :],
                         rhs=code_embT[:], start=True, stop=False,
                         skip_group_check=True)
        nc.tensor.matmul(out=fp[:], lhsT=finw_hi[:],
                         rhs=trans_embT[:], start=False, stop=True,
                         skip_group_check=True)
        f_s = const.tile([FIN, GPC], F32, tag="f_s")
        nc.scalar.activation(out=f_s[:], in_=fp[:], func=AF.Identity,
                             bias=ws["fin_b"][:, :1])
        ef = const.tile([FIN, GPC], F32, tag="ef")
        nc.scalar.activation(out=ef[:], in_=f_s[:], func=AF.Exp)
        lfp = pp.tile([1, GPC], F32, tag="up")
        nc.tensor.matmul(out=lfp[:], lhsT=ones_f1[:], rhs=ef[:],
                         start=True, stop=True)
        lf_s = const.tile([1, GPC], F32, tag="lf_s")
        nc.scalar.activation(out=lf_s[:], in_=lfp[:], func=AF.Ln)
        bfp = pp3.tile([FIN, GPC], F32, tag="zp")
        nc.tensor.matmul(out=bfp[:], lhsT=ones_1f[:], rhs=lf_s[:],
                         start=True, stop=True)
        outT = const.tile([FIN, GPC], F32, tag="outT")
        nc.vector.tensor_tensor(out=outT[:], in0=f_s[:], in1=bfp[:],
                                op=ALU.subtract)
        nc.sync.dma_start(out=out_d[:], in_=outT[:])

    if not nc.is_finalized():
        nc.finalize()
    return nc


# ---------------------------------------------------------------- runner

def make_in_maps(inputs, plan, cfg):
    c = cfg
    wnames = ["c1_w1", "c1_b1", "c1_gamma", "c1_beta", "c1_w2", "c1_b2",
              "c2_w1", "c2_b1", "c2_gamma", "c2_beta", "c2_w2", "c2_b2",
              "g_l1_w", "g_l1_b", "g_l2_w", "g_l2_b",
              "fc1_w", "fc1_b", "fc2_w", "fc2_b", "fc3_w", "fc3_b",
              "fin_w", "fin_b"]
    np_gdt = np.float32 if c.GDT == F32 else __import__("ml_dtypes").bfloat16
    x = np.asarray(inputs["x"], np.float32)
    xg = x.astype(np_gdt)
    S = plan["S"]
    node_lo = plan["node_lo"]
    code = np.ascontiguousarray(np.asarray(inputs["code_x"], np.float32))
    ident = np.eye(128, dtype=np.float32)
    b2r = np.ascontiguousarray(
        np.asarray(inputs["c2_b2"], np.float32).reshape(1, c.D))
    in_maps = []
    for ci in range(c.W):
        lo, hi = int(node_lo[ci]), int(node_lo[ci + 1])
        xownT = np.zeros((128, S), np.float32)
        xownT[:, :hi - lo] = x[lo:hi].T
        m = {
            "xg": xg,
            "xownT": xownT,
            "pmat": plan["pmats"][ci],
            "code": code[ci * c.GPC:(ci + 1) * c.GPC],
            "ident": ident,
            "nh": np.full((128, 1), float(S - plan["n_real"][ci]), np.float32),
            "c2_b2r": b2r,
        }
        for li in (0, 1):
            cv = plan["conv"][li]
            m[f"idx{li}"] = cv["cores"][ci]["idx16"]
            m[f"col{li}"] = cv["cores"][ci]["colidx"].astype(np_gdt)
        for k in wnames:
            m[k] = np.ascontiguousarray(np.asarray(inputs[k], np.float32))
        in_maps.append(m)
    return in_maps


_CACHE = {}


def _get_compiled(inputs, cfg):
    if "prog" not in _CACHE:
        plan = _plan(inputs["edge_index"], inputs["batch"], cfg)
        nc = _build(plan, cfg)
        _CACHE["prog"] = (plan, nc)
    return _CACHE["prog"]


def kernel(**inputs) -> np.ndarray:
    cfg = DEFAULT_CFG
    plan, nc = _get_compiled(inputs, cfg)
    in_maps = make_in_maps(inputs, plan, cfg)
    res = run_bass_kernel_spmd(nc, in_maps, core_ids=list(range(cfg.W)))
    outs = [res.results[ci]["out"].T for ci in range(cfg.W)]
    return np.ascontiguousarray(np.concatenate(outs, axis=0).astype(np.float32))
